# revision 57
# baseline (speedup 1.0000x reference)
"""Trainium2 Bass kernel: 2-layer GCN (PyG-style GCNConv x2) on 8 NeuronCores.

Strategy:
  - Nodes sharded contiguously across 8 cores (12500 rows each).
  - Per layer: dense h' = (x @ W) * dinv[row] computed on the owning core
    (bf16) into a resident SBUF tile (hl), dumped to DRAM in one contiguous
    DMA (tile-major layout), AllGather'd to every core (25.6MB replica),
    then per-core sparse aggregation over its in-edges:
      gather h'[src] rows via dma_gather (int16 idx into 4 blocks of 25088
      storage positions; 4 SWDGE queues round-robin so Q7 descriptor
      emission runs on all four core pairs concurrently; per-core exact
      counts via num_idxs_reg + trailing -1 idx so padding is never
      gathered; per-bucket idx sorted ascending for HBM row locality),
      scatter-add via one-hot matmul into PSUM per 128-dst tile,
      self-loops as a diagonal identity matmul from the resident hl tile,
      bias added as rank-1 matmul outer(sqrt(deg), b),
      eviction scaled by dinv[dst] on the scalar engine (layer 1 fuses the
      layer-2 dense transform behind a PE transpose).
  - The per-edge norm dinv[src]*dinv[dst] is folded into the two node-level
    scalings, so no per-edge multiply exists anywhere.
  - All matmul/gather traffic in bf16 (tolerance 2e-2 >> bf16 error);
    PSUM accumulation fp32; final output fp32.
"""

import os
import sys

for _p in ("/opt/trn_rl_repo",):
    if _p not in sys.path:
        sys.path.append(_p)

import numpy as np
import ml_dtypes

import concourse.bacc as bacc
import concourse.bass as bass
import concourse.mybir as mybir
import concourse.tile as tile
from concourse.bass_utils import run_bass_kernel_spmd

F32 = mybir.dt.float32
BF16 = mybir.dt.bfloat16
I16 = mybir.dt.int16
AF = mybir.ActivationFunctionType
ALU = mybir.AluOpType
NPBF = ml_dtypes.bfloat16

N_NODES = 100000
D = 128
NCORES = 8
TILE = 128
NQ = 4  # SWDGE queues round-robin for dma_gather descriptor emission


def _ceil_div(a, b):
    return (a + b - 1) // b


class Plan:
    """Core-uniform structure tables derived from the edge index."""

    def __init__(self, n_nodes, edge_index, group_tiles=4):
        self.n = n_nodes
        self.ns = n_nodes // NCORES            # nodes per core
        self.nt = _ceil_div(self.ns, TILE)     # dst tiles per core
        self.last_w = self.ns - (self.nt - 1) * TILE
        self.G = group_tiles
        # h' is stored tile-major per core and AllGather'd in two halves
        # (tiles < nth, tiles >= nth) so each AG overlaps surrounding work.
        # storage pos of node (c, t, rr): half*NH + (c*128+rr)*nth + (t%nth)
        self.nth = self.nt // 2                # tiles per half (49)
        self.NH = NCORES * TILE * self.nth     # positions per half
        self.npos = 2 * self.NH
        self.nblk = 4                          # src blocks (int16 idx limit)
        self.blk = self.npos // self.nblk      # 2 blocks per half

        # degree includes self-loops (PyG GCNConv semantics) but the self
        # edges are NOT gathered: handled as a diagonal identity matmul.
        deg = np.bincount(edge_index[1], minlength=n_nodes).astype(np.float32)
        deg += 1.0
        self.dinv = deg ** -0.5
        self.sdeg = np.sqrt(deg)
        src = np.asarray(edge_index[0])
        dst = np.asarray(edge_index[1])

        core = dst // self.ns
        tloc = (dst % self.ns) // TILE
        sc = src // self.ns
        sr = src % self.ns
        st = sr // TILE
        half = (st >= self.nth).astype(np.int64)
        pos = (half * self.NH + (sc * TILE + (sr % TILE)) * self.nth
               + (st - half * self.nth))
        blk = pos // self.blk
        key = (core * self.nt + tloc) * self.nblk + blk
        # ascending storage pos within each (core,tile,blk) bucket: the
        # per-bucket gather descriptors then sweep HBM monotonically
        order = np.lexsort((pos, key))
        self.pos_s = pos[order]
        self.doff_s = ((dst % self.ns) % TILE)[order]
        counts = np.bincount(key, minlength=NCORES * self.nt * self.nblk)
        self.counts = counts.reshape(NCORES, self.nt, self.nblk)
        # segment start offsets into src_s per (core, tile, blk)
        self.seg_off = np.zeros(NCORES * self.nt * self.nblk + 1, dtype=np.int64)
        np.cumsum(counts, out=self.seg_off[1:])

        # chunks per (tile, blk): shared across cores
        self.CT = _ceil_div(self.counts, TILE).max(axis=0)  # [nt, nblk]

        # tile groups
        self.groups = [list(range(g, min(g + self.G, self.nt)))
                       for g in range(0, self.nt, self.G)]

        # per (group, blk): chunk count and the (tile, n_chunks) layout
        self.gb_chunks = []   # [g][b] -> list of (tile, CT[t][b])
        self.gb_C = []        # [g][b] -> total chunks
        for tiles in self.groups:
            row_l, row_c = [], []
            for b in range(self.nblk):
                lay = [(t, int(self.CT[t, b])) for t in tiles if self.CT[t, b] > 0]
                row_l.append(lay)
                row_c.append(sum(c for _, c in lay))
            self.gb_chunks.append(row_l)
            self.gb_C.append(row_c)

        # dma_gather call schedule: one call never spans a (tile, blk)
        # segment so per-core trailing -1 idx trimming (num_idxs_reg) works
        self.GMAX = 8
        self.calls = []   # (g, b, t, kb: chunk col in bucket, c0: within seg, cn)
        for g in range(len(self.groups)):
            for b in range(self.nblk):
                kb = 0
                for (t, nch) in self.gb_chunks[g][b]:
                    for c0 in range(0, nch, self.GMAX):
                        cn = min(self.GMAX, nch - c0)
                        self.calls.append((g, b, t, kb, c0, cn))
                    kb += nch
        self.ncalls = len(self.calls)
        # tight per-call num_idxs immediate: the max real count over cores,
        # rounded to 16 (the ucode's idx-unpack cost scales with it)
        self.call_imm = []
        for (g, b, t, kb, c0, cn) in self.calls:
            mx = int(self.counts[:, t, b].max()) - c0 * TILE
            mx = min(max(mx, 1), cn * TILE)
            imm = min(((mx + 15) // 16) * 16, cn * TILE)
            assert imm > (cn - 1) * TILE, (imm, cn)
            self.call_imm.append(imm)

        # column offsets in the concatenated idx / dstoff DRAM buffers
        self.idx_col = []     # [g][b] -> start col in idx buffer (int16, /16 wrap)
        self.ch_col = []      # [g] -> start chunk col in dstoff buffer
        ic = 0
        cc = 0
        for g in range(len(self.groups)):
            self.ch_col.append(cc)
            row = []
            for b in range(self.nblk):
                row.append(ic)
                ic += self.gb_C[g][b] * (TILE // 16)
                cc += self.gb_C[g][b]
            self.idx_col.append(row)
        self.idx_cols = ic
        self.ch_cols = cc

    def core_inputs(self, c):
        """Build idx (int16, -1 pad), dstoff (bf16) and per-call counts."""
        idx = np.full((16, self.idx_cols), -1, dtype=np.int16)
        doff = np.full((128, self.ch_cols), -1.0, dtype=np.float32)
        for g, tiles in enumerate(self.groups):
            ch = self.ch_col[g]
            for b in range(self.nblk):
                icol = self.idx_col[g][b]
                for (t, nch) in self.gb_chunks[g][b]:
                    cnt = int(self.counts[c, t, b])
                    o = self.seg_off[(c * self.nt + t) * self.nblk + b]
                    nslots = nch * TILE
                    a = np.full(nslots, -1, dtype=np.int16)
                    a[:cnt] = (self.pos_s[o:o + cnt] - b * self.blk).astype(np.int16)
                    idx[:, icol:icol + nch * 8] = a.reshape(nch * 8, 16).T
                    dv = np.full(nslots, -1.0, dtype=np.float32)
                    dv[:cnt] = self.doff_s[o:o + cnt].astype(np.float32)
                    doff[:, ch:ch + nch] = dv.reshape(nch, 128).T
                    icol += nch * 8
                    ch += nch
        idx_full = np.tile(idx, (8, 1))
        cnts = np.zeros((1, self.ncalls), dtype=np.int32)
        for i, (g, b, t, kb, c0, cn) in enumerate(self.calls):
            cnt = int(self.counts[c, t, b])
            cnts[0, i] = min(max(cnt - c0 * TILE, 0), cn * TILE)
        return idx_full, doff.astype(NPBF), cnts


def _build(plan):
    """Build the SPMD bass program (shared by all 8 cores)."""
    n, ns, nt, nblk, blk = plan.n, plan.ns, plan.nt, plan.nblk, plan.blk
    nc = bacc.Bacc("TRN2", target_bir_lowering=False, debug=False,
                   num_devices=NCORES, num_swdge_queues=NQ)

    I32 = mybir.dt.int32
    xT = nc.dram_tensor("xT", [D, ns], BF16, kind="ExternalInput").ap()
    cnts_d = nc.dram_tensor("cnts", [1, plan.ncalls], I32,
                            kind="ExternalInput").ap()
    wts = nc.dram_tensor("wts", [D, 2 * D], BF16, kind="ExternalInput").ap()
    consts = nc.dram_tensor("consts", [D, 2 * D], BF16, kind="ExternalInput").ap()
    brow = nc.dram_tensor("brow", [1, 2 * D], BF16, kind="ExternalInput").ap()
    dinv_c = nc.dram_tensor("dinv_c", [D, nt], F32, kind="ExternalInput").ap()
    sdeg_r = nc.dram_tensor("sdeg_r", [1, nt * TILE], BF16, kind="ExternalInput").ap()
    idx_d = nc.dram_tensor("idx", [D, plan.idx_cols], I16, kind="ExternalInput").ap()
    doff_d = nc.dram_tensor("doff", [D, plan.ch_cols], BF16, kind="ExternalInput").ap()
    out_d = nc.dram_tensor("out", [ns, D], F32, kind="ExternalOutput").ap()

    # tile-major storage, split in two tile-halves per layer; each half is
    # dumped contiguously and AllGather'd separately (overlaps compute)
    nth = plan.nth
    hbh = [[nc.dram_tensor(f"h{i}b{h}", [TILE, nth * D], BF16).ap()
            for h in range(2)] for i in range(2)]
    hfh = [[nc.dram_tensor(f"h{i}f{h}", [plan.NH, D], BF16,
                           addr_space="Shared").ap()
            for h in range(2)] for i in range(2)]

    max_C = max(sum(plan.gb_C[g]) for g in range(len(plan.groups)))
    max_icols = max(sum(plan.gb_C[g]) * 8 for g in range(len(plan.groups)))

    with tile.TileContext(nc) as tc:
        with (
            tc.tile_pool(name="const", bufs=1) as cpool,
            tc.tile_pool(name="xstream", bufs=3) as xpool,
            tc.tile_pool(name="stage", bufs=14) as spool,
            tc.tile_pool(name="oh", bufs=3) as ohpool,
            tc.tile_pool(name="aux", bufs=8) as auxpool,
            tc.tile_pool(name="ev", bufs=4) as evpool,
            tc.tile_pool(name="acc", bufs=4, space="PSUM") as accpool,
            tc.tile_pool(name="ptr", bufs=2, space="PSUM") as trpool,
            tc.tile_pool(name="pd", bufs=2, space="PSUM") as pdpool,
        ):
            w_sb = cpool.tile([D, 2 * D], BF16, tag="w")
            nc.sync.dma_start(out=w_sb[:], in_=wts[:])
            co_sb = cpool.tile([D, 2 * D], BF16, tag="co")
            nc.sync.dma_start(out=co_sb[:], in_=consts[:])
            br_sb = cpool.tile([1, 2 * D], BF16, tag="br")
            nc.sync.dma_start(out=br_sb[:], in_=brow[:])
            dv_sb = cpool.tile([D, nt], F32, tag="dv")
            nc.sync.dma_start(out=dv_sb[:], in_=dinv_c[:])
            sd_sb = cpool.tile([1, nt * TILE], BF16, tag="sd")
            nc.sync.dma_start(out=sd_sb[:], in_=sdeg_r[:])
            cn_sb = cpool.tile([1, plan.ncalls], mybir.dt.int32, tag="cn")
            nc.sync.dma_start(out=cn_sb[:], in_=cnts_d[:])
            # local h' tiles, resident for the diagonal (self-loop) matmul
            hl = [cpool.tile([TILE, nt, D], BF16, tag=f"hl{i}", name=f"hl{i}")
                  for i in range(2)]
            gregs = [nc.gpsimd.alloc_register(f"gcnt{i}") for i in range(NQ)]

            W1 = w_sb[:, 0:D]
            W2 = w_sb[:, D:2 * D]
            iota = co_sb[:, 0:D]
            ident = co_sb[:, D:2 * D]

            def tw(t):
                return TILE if t < nt - 1 else plan.last_w

            def dump_half(li, h):
                """hl[li] tile-half h -> hbh[li][h], then AllGather it."""
                nc.sync.dma_start(
                    out=hbh[li][h][:],
                    in_=hl[li][:, h * nth:(h + 1) * nth, :].opt())
                nc.gpsimd.collective_compute(
                    "AllGather", ALU.bypass,
                    replica_groups=[list(range(NCORES))],
                    ins=[hbh[li][h].opt()], outs=[hfh[li][h].opt()])

            # ---- layer-1 dense: h0' = (x @ W1) * dinv ----
            SLAB = 8
            for s in range(0, nt, SLAB):
                sn = min(SLAB, nt - s)
                sw = (sn - 1) * TILE + tw(s + sn - 1)
                xt_t = xpool.tile([D, SLAB * TILE], BF16, tag="xt")
                nc.sync.dma_start(out=xt_t[:, :sw],
                                  in_=xT[:, s * TILE:s * TILE + sw])
                for j in range(sn):
                    t = s + j
                    w = tw(t)
                    pd = pdpool.tile([TILE, D], F32, tag="pd")
                    nc.tensor.matmul(pd[:w, :],
                                     lhsT=xt_t[:, j * TILE:j * TILE + w],
                                     rhs=W1, start=True, stop=True)
                    nc.scalar.activation(hl[0][:w, t, :], pd[:w, :], AF.Copy,
                                         scale=dv_sb[:w, t:t + 1])
                if s <= nth - 1 < s + sn:
                    dump_half(0, 0)   # first tile-half done -> AG overlaps rest
            dump_half(0, 1)

            # ---- sparse layer (templated over layer index) ----
            max_Cgb = max((plan.gb_C[g][b] for g in range(len(plan.groups))
                           for b in range(nblk)), default=1)

            # zero all stage buffers once: trimmed gathers leave untouched
            # slots whose virgin SBUF content may be NaN bit patterns, and
            # the PE turns 0 * NaN into NaN despite the zero one-hot column
            for _i in range(14):
                stg0 = spool.tile([D, max_Cgb, TILE], BF16, tag="stage",
                                  name="stg")
                nc.vector.memset(stg0[:].opt(), 0.0)

            qctr = [0]

            # per (g, b) -> list of (t, kb, c0, cn, call_idx)
            bucket_calls = {}
            for ci, (g, b, t, kb, c0, cn) in enumerate(plan.calls):
                bucket_calls.setdefault((g, b), []).append((t, kb, c0, cn, ci))

            # group index after which hl[li=1] tiles 0..nth-1 are all evicted
            g_half = next(gi for gi, tiles in enumerate(plan.groups)
                          if tiles[-1] >= nth - 1)

            def sparse_layer(li):
                tbl = {}       # g -> (idx_sb, do_sb)
                stgs = {}      # (g, b) -> gathered stage tile
                accs_map = {}  # g -> accs

                def load_tables(g):
                    Ctot = sum(plan.gb_C[g])
                    icols = Ctot * 8
                    idx_sb = auxpool.tile([D, max_icols], I16, tag="idx",
                                          name="idx_sb")
                    nc.sync.dma_start(
                        out=idx_sb[:, :icols],
                        in_=idx_d[:, plan.idx_col[g][0]:plan.idx_col[g][0] + icols])
                    do_sb = auxpool.tile([D, max_C], BF16, tag="doff",
                                         name="do_sb")
                    nc.sync.dma_start(
                        out=do_sb[:, :Ctot],
                        in_=doff_d[:, plan.ch_col[g]:plan.ch_col[g] + Ctot])
                    tbl[g] = (idx_sb, do_sb)

                def init_accs(g):
                    accs = {}
                    for t in plan.groups[g]:
                        w = tw(t)
                        accs[t] = accpool.tile([TILE, D], F32, tag="acc", name=f"acc_t{t}")
                        # diagonal (self-loop) term: acc = I @ h'[tile]
                        nc.tensor.matmul(
                            accs[t][:w, :], lhsT=ident[:w, :w],
                            rhs=hl[li][:w, t, :], start=True, stop=False)
                    accs_map[g] = accs

                def do_gather(g, b):
                    idx_sb, _ = tbl[g]
                    Cgb = plan.gb_C[g][b]
                    if Cgb == 0:
                        return
                    ic0 = plan.idx_col[g][b] - plan.idx_col[g][0]
                    stg = spool.tile([D, max_Cgb, TILE], BF16, tag="stage",
                                     name="stg")
                    src_half = hfh[li][b // 2]
                    sbase = (b % 2) * blk
                    for (t, kb, c0, cn, ci) in bucket_calls[(g, b)]:
                        q = qctr[0] % NQ
                        nc.gpsimd.reg_load(gregs[q], cn_sb[0:1, ci:ci + 1])
                        nc.gpsimd.dma_gather(
                            stg[:, kb + c0:kb + c0 + cn, :],
                            src_half[sbase:sbase + blk, :],
                            idx_sb[:, ic0 + (kb + c0) * 8:
                                   ic0 + (kb + c0 + cn) * 8],
                            plan.call_imm[ci],
                            gregs[q],
                            D,
                            queue_num=q,
                        )
                        qctr[0] += 1
                    stgs[(g, b)] = stg

                def do_mm(g, b):
                    Cgb = plan.gb_C[g][b]
                    if Cgb == 0:
                        return
                    _, do_sb = tbl[g]
                    accs = accs_map[g]
                    stg = stgs.pop((g, b))
                    gco = sum(plan.gb_C[g][:b])
                    oh_sb = ohpool.tile([D, max_Cgb, TILE], BF16, tag="oh",
                                        name="oh_sb")
                    nc.vector.scalar_tensor_tensor(
                        out=oh_sb[:, :Cgb, :],
                        in0=do_sb[:, gco:gco + Cgb].unsqueeze(2)
                            .broadcast_to([D, Cgb, TILE]),
                        scalar=1.0,
                        in1=iota.unsqueeze(1).broadcast_to([D, Cgb, TILE]),
                        op0=ALU.mult,
                        op1=ALU.is_equal,
                    )
                    k = 0
                    for (t, nch) in plan.gb_chunks[g][b]:
                        for _ in range(nch):
                            nc.tensor.matmul(
                                accs[t][:], lhsT=oh_sb[:, k, :],
                                rhs=stg[:, k, :],
                                start=False, stop=False)
                            k += 1

                def finish_group(g, li):
                    tiles = plan.groups[g]
                    accs = accs_map[g]
                    for t in tiles:
                        w = tw(t)
                        acc = accs[t]
                        # bias as rank-1: outer(sqrt(deg), b); sdeg rows
                        # beyond the tile width are zero-padded on the host
                        nc.tensor.matmul(
                            acc[:],
                            lhsT=sd_sb[:, t * TILE:(t + 1) * TILE],
                            rhs=br_sb[:, li * D:(li + 1) * D],
                            start=False, stop=True)
                        if li == 0:
                            ev = evpool.tile([TILE, D], BF16, tag="ev")
                            nc.scalar.activation(ev[:w, :], acc[:w, :], AF.Copy,
                                                 scale=dv_sb[:w, t:t + 1])
                            # fused layer-2 dense: h1' = (out1 @ W2) * dinv
                            ptr = trpool.tile([D, TILE], BF16, tag="ptr")
                            nc.tensor.transpose(ptr[:, :w], ev[:w, :],
                                                ident[:w, :w])
                            trs = evpool.tile([D, TILE], BF16, tag="trs")
                            nc.vector.tensor_copy(trs[:, :w], ptr[:, :w])
                            pd = pdpool.tile([TILE, D], F32, tag="pd")
                            nc.tensor.matmul(pd[:w, :], lhsT=trs[:, :w], rhs=W2,
                                             start=True, stop=True)
                            nc.scalar.activation(hl[1][:w, t, :], pd[:w, :],
                                                 AF.Copy,
                                                 scale=dv_sb[:w, t:t + 1])
                        else:
                            evf = evpool.tile([TILE, D], F32, tag="evf")
                            nc.scalar.activation(evf[:w, :], acc[:w, :], AF.Copy,
                                                 scale=dv_sb[:w, t:t + 1])
                            nc.sync.dma_start(
                                out=out_d[t * TILE:t * TILE + w, :],
                                in_=evf[:w, :])

                    del accs_map[g]
                    if li == 0 and g == g_half:
                        # first tile-half of h1' is complete: AG it now so it
                        # overlaps the rest of sparse layer 1
                        dump_half(1, 0)

                # first K groups: gather blocks 0-1 (first-half AG) before any
                # block 2-3 gather, so the second-half AG completes under work
                ngroups = len(plan.groups)
                K = min(6, ngroups)
                for g in range(K):
                    load_tables(g)
                    do_gather(g, 0)
                    do_gather(g, 1)
                for g in range(K):
                    init_accs(g)
                    do_mm(g, 0)
                    do_mm(g, 1)
                    do_gather(g, 2)
                    do_mm(g, 2)
                    do_gather(g, 3)
                    do_mm(g, 3)
                    finish_group(g, li)
                for g in range(K, ngroups):
                    load_tables(g)
                    init_accs(g)
                    for b in range(nblk):
                        do_gather(g, b)
                        do_mm(g, b)
                    finish_group(g, li)

            sparse_layer(0)
            dump_half(1, 1)
            sparse_layer(1)

    nc.compile()
    return nc


def _install_ntff_hook():
    """antenv.axon_hooks is absent in this image; synthesize it and register
    the ctypes NTFF profile hook from the boot module."""
    import types
    if "antenv.axon_hooks" in sys.modules:
        return
    try:
        from trn_agent_boot.trn_boot import _ntff_profile_via_ctypes
        hook = _ntff_profile_via_ctypes("/opt/axon/libaxon_pjrt.so")
    except Exception as e:
        print(f"[kernel] ntff hook unavailable: {e}", flush=True)
        hook = None
    mod = types.ModuleType("antenv.axon_hooks")
    mod._hook = hook
    mod.set_axon_ntff_profile_hook = lambda h: setattr(mod, "_hook", h)
    mod.get_axon_ntff_profile_hook = lambda: mod._hook
    sys.modules["antenv.axon_hooks"] = mod
    import antenv
    antenv.axon_hooks = mod


def _run(plan, x, W1, b1, W2, b2, trace=False, stage="full"):
    import time
    if trace:
        _install_ntff_hook()
    t0 = time.time()
    nc = _build(plan)
    t1 = time.time()
    if os.environ.get("GCN_VERBOSE"):
        print(f"[kernel] build+compile: {t1 - t0:.1f}s", flush=True)
    ns, nt = plan.ns, plan.nt
    iota_t = np.tile(np.arange(TILE, dtype=np.float32), (TILE, 1))
    ident_t = np.eye(TILE, dtype=np.float32)
    consts = np.concatenate([iota_t, ident_t], axis=1).astype(NPBF)
    wts = np.concatenate([W1.astype(np.float32), W2.astype(np.float32)],
                         axis=1).astype(NPBF)
    brow = np.concatenate([b1.astype(np.float32), b2.astype(np.float32)]
                          ).reshape(1, 2 * D).astype(NPBF)

    in_maps = []
    for c in range(NCORES):
        lo, hi = c * ns, (c + 1) * ns
        dv = plan.dinv[lo:hi]
        # column t of dcol holds dinv[lo + t*128 : lo + (t+1)*128] (pad 1.0)
        dcol = np.ones((nt, TILE), dtype=np.float32)
        dcol.reshape(-1)[:ns] = dv
        dcol = np.ascontiguousarray(dcol.T)
        sdr = np.zeros((1, nt * TILE), dtype=np.float32)
        sdr[0, :ns] = plan.sdeg[lo:hi]
        idx, doff, cnts = plan.core_inputs(c)
        in_maps.append({
            "xT": np.ascontiguousarray(x[lo:hi].astype(np.float32).T).astype(NPBF),
            "wts": wts, "consts": consts, "brow": brow,
            "dinv_c": dcol, "sdeg_r": sdr.astype(NPBF),
            "idx": idx, "doff": doff, "cnts": cnts,
        })
    t2 = time.time()
    res = run_bass_kernel_spmd(nc, in_maps, core_ids=list(range(NCORES)),
                               trace=trace)
    if os.environ.get("GCN_VERBOSE"):
        print(f"[kernel] prep inputs: {t2 - t1:.1f}s, run: {time.time() - t2:.1f}s",
              flush=True)
    out = np.concatenate([res.results[c]["out"] for c in range(NCORES)], axis=0)
    return out, res


def kernel(x, edge_index, W1, b1, W2, b2):
    plan = Plan(x.shape[0], np.asarray(edge_index))
    out, _ = _run(plan, np.asarray(x), np.asarray(W1), np.asarray(b1),
                  np.asarray(W2), np.asarray(b2))
    return out


# revision 59
# speedup vs baseline: 1.0018x; 1.0018x over previous
"""Trainium2 Bass kernel: 2-layer GCN (PyG-style GCNConv x2) on 8 NeuronCores.

Strategy:
  - Nodes sharded contiguously across 8 cores (12500 rows each).
  - Per layer: dense h' = (x @ W) * dinv[row] computed on the owning core
    (bf16) into a resident SBUF tile (hl), dumped to DRAM in one contiguous
    DMA (tile-major layout), AllGather'd to every core (25.6MB replica),
    then per-core sparse aggregation over its in-edges:
      gather h'[src] rows via dma_gather (int16 idx into 4 blocks of 25088
      storage positions; 4 SWDGE queues round-robin so Q7 descriptor
      emission runs on all four core pairs concurrently; per-core exact
      counts via num_idxs_reg + trailing -1 idx so padding is never
      gathered; per-bucket idx sorted ascending for HBM row locality),
      scatter-add via one-hot matmul into PSUM per 128-dst tile,
      self-loops as a diagonal identity matmul from the resident hl tile,
      bias added as rank-1 matmul outer(sqrt(deg), b),
      eviction scaled by dinv[dst] on the scalar engine (layer 1 fuses the
      layer-2 dense transform behind a PE transpose).
  - The per-edge norm dinv[src]*dinv[dst] is folded into the two node-level
    scalings, so no per-edge multiply exists anywhere.
  - All matmul/gather traffic in bf16 (tolerance 2e-2 >> bf16 error);
    PSUM accumulation fp32; final output fp32.
"""

import os
import sys

for _p in ("/opt/trn_rl_repo",):
    if _p not in sys.path:
        sys.path.append(_p)

import numpy as np
import ml_dtypes

import concourse.bacc as bacc
import concourse.bass as bass
import concourse.mybir as mybir
import concourse.tile as tile
from concourse.bass_utils import run_bass_kernel_spmd

F32 = mybir.dt.float32
BF16 = mybir.dt.bfloat16
I16 = mybir.dt.int16
AF = mybir.ActivationFunctionType
ALU = mybir.AluOpType
NPBF = ml_dtypes.bfloat16

N_NODES = 100000
D = 128
NCORES = 8
TILE = 128
NQ = 4  # SWDGE queues round-robin for dma_gather descriptor emission


def _ceil_div(a, b):
    return (a + b - 1) // b


class Plan:
    """Core-uniform structure tables derived from the edge index."""

    def __init__(self, n_nodes, edge_index, group_tiles=4):
        self.n = n_nodes
        self.ns = n_nodes // NCORES            # nodes per core
        self.nt = _ceil_div(self.ns, TILE)     # dst tiles per core
        self.last_w = self.ns - (self.nt - 1) * TILE
        self.G = group_tiles
        # h' is stored tile-major per core and AllGather'd in two halves
        # (tiles < nth, tiles >= nth) so each AG overlaps surrounding work.
        # storage pos of node (c, t, rr): half*NH + (c*128+rr)*nth + (t%nth)
        self.nth = self.nt // 2                # tiles per half (49)
        self.NH = NCORES * TILE * self.nth     # positions per half
        self.npos = 2 * self.NH
        self.nblk = 4                          # src blocks (int16 idx limit)
        self.blk = self.npos // self.nblk      # 2 blocks per half

        # degree includes self-loops (PyG GCNConv semantics) but the self
        # edges are NOT gathered: handled as a diagonal identity matmul.
        deg = np.bincount(edge_index[1], minlength=n_nodes).astype(np.float32)
        deg += 1.0
        self.dinv = deg ** -0.5
        self.sdeg = np.sqrt(deg)
        src = np.asarray(edge_index[0])
        dst = np.asarray(edge_index[1])

        core = dst // self.ns
        tloc = (dst % self.ns) // TILE
        sc = src // self.ns
        sr = src % self.ns
        st = sr // TILE
        half = (st >= self.nth).astype(np.int64)
        pos = (half * self.NH + (sc * TILE + (sr % TILE)) * self.nth
               + (st - half * self.nth))
        blk = pos // self.blk
        key = (core * self.nt + tloc) * self.nblk + blk
        # ascending storage pos within each (core,tile,blk) bucket: the
        # per-bucket gather descriptors then sweep HBM monotonically
        order = np.lexsort((pos, key))
        self.pos_s = pos[order]
        self.doff_s = ((dst % self.ns) % TILE)[order]
        counts = np.bincount(key, minlength=NCORES * self.nt * self.nblk)
        self.counts = counts.reshape(NCORES, self.nt, self.nblk)
        # segment start offsets into src_s per (core, tile, blk)
        self.seg_off = np.zeros(NCORES * self.nt * self.nblk + 1, dtype=np.int64)
        np.cumsum(counts, out=self.seg_off[1:])

        # chunks per (tile, blk): shared across cores
        self.CT = _ceil_div(self.counts, TILE).max(axis=0)  # [nt, nblk]

        # tile groups
        self.groups = [list(range(g, min(g + self.G, self.nt)))
                       for g in range(0, self.nt, self.G)]

        # per (group, blk): chunk count and the (tile, n_chunks) layout
        self.gb_chunks = []   # [g][b] -> list of (tile, CT[t][b])
        self.gb_C = []        # [g][b] -> total chunks
        for tiles in self.groups:
            row_l, row_c = [], []
            for b in range(self.nblk):
                lay = [(t, int(self.CT[t, b])) for t in tiles if self.CT[t, b] > 0]
                row_l.append(lay)
                row_c.append(sum(c for _, c in lay))
            self.gb_chunks.append(row_l)
            self.gb_C.append(row_c)

        # dma_gather call schedule: one call never spans a (tile, blk)
        # segment so per-core trailing -1 idx trimming (num_idxs_reg) works
        self.GMAX = 8
        self.calls = []   # (g, b, t, kb: chunk col in bucket, c0: within seg, cn)
        for g in range(len(self.groups)):
            for b in range(self.nblk):
                kb = 0
                for (t, nch) in self.gb_chunks[g][b]:
                    for c0 in range(0, nch, self.GMAX):
                        cn = min(self.GMAX, nch - c0)
                        self.calls.append((g, b, t, kb, c0, cn))
                    kb += nch
        self.ncalls = len(self.calls)
        # tight per-call num_idxs immediate: the max real count over cores,
        # rounded to 16 (the ucode's idx-unpack cost scales with it)
        self.call_imm = []
        for (g, b, t, kb, c0, cn) in self.calls:
            mx = int(self.counts[:, t, b].max()) - c0 * TILE
            mx = min(max(mx, 1), cn * TILE)
            imm = min(((mx + 15) // 16) * 16, cn * TILE)
            assert imm > (cn - 1) * TILE, (imm, cn)
            self.call_imm.append(imm)

        # column offsets in the concatenated idx / dstoff DRAM buffers
        self.idx_col = []     # [g][b] -> start col in idx buffer (int16, /16 wrap)
        self.ch_col = []      # [g] -> start chunk col in dstoff buffer
        ic = 0
        cc = 0
        for g in range(len(self.groups)):
            self.ch_col.append(cc)
            row = []
            for b in range(self.nblk):
                row.append(ic)
                ic += self.gb_C[g][b] * (TILE // 16)
                cc += self.gb_C[g][b]
            self.idx_col.append(row)
        self.idx_cols = ic
        self.ch_cols = cc

    def core_inputs(self, c):
        """Build idx (int16, -1 pad), dstoff (bf16) and per-call counts."""
        idx = np.full((16, self.idx_cols), -1, dtype=np.int16)
        doff = np.full((128, self.ch_cols), -1.0, dtype=np.float32)
        for g, tiles in enumerate(self.groups):
            ch = self.ch_col[g]
            for b in range(self.nblk):
                icol = self.idx_col[g][b]
                for (t, nch) in self.gb_chunks[g][b]:
                    cnt = int(self.counts[c, t, b])
                    o = self.seg_off[(c * self.nt + t) * self.nblk + b]
                    nslots = nch * TILE
                    a = np.full(nslots, -1, dtype=np.int16)
                    a[:cnt] = (self.pos_s[o:o + cnt] - b * self.blk).astype(np.int16)
                    idx[:, icol:icol + nch * 8] = a.reshape(nch * 8, 16).T
                    dv = np.full(nslots, -1.0, dtype=np.float32)
                    dv[:cnt] = self.doff_s[o:o + cnt].astype(np.float32)
                    doff[:, ch:ch + nch] = dv.reshape(nch, 128).T
                    icol += nch * 8
                    ch += nch
        idx_full = np.tile(idx, (8, 1))
        cnts = np.zeros((1, self.ncalls), dtype=np.int32)
        for i, (g, b, t, kb, c0, cn) in enumerate(self.calls):
            cnt = int(self.counts[c, t, b])
            cnts[0, i] = min(max(cnt - c0 * TILE, 0), cn * TILE)
        return idx_full, doff.astype(NPBF), cnts


def _build(plan):
    """Build the SPMD bass program (shared by all 8 cores)."""
    n, ns, nt, nblk, blk = plan.n, plan.ns, plan.nt, plan.nblk, plan.blk
    nc = bacc.Bacc("TRN2", target_bir_lowering=False, debug=False,
                   num_devices=NCORES, num_swdge_queues=NQ)

    I32 = mybir.dt.int32
    xT = nc.dram_tensor("xT", [D, ns], BF16, kind="ExternalInput").ap()
    cnts_d = nc.dram_tensor("cnts", [1, plan.ncalls], I32,
                            kind="ExternalInput").ap()
    wts = nc.dram_tensor("wts", [D, 2 * D], BF16, kind="ExternalInput").ap()
    consts = nc.dram_tensor("consts", [D, 2 * D], BF16, kind="ExternalInput").ap()
    brow = nc.dram_tensor("brow", [1, 2 * D], BF16, kind="ExternalInput").ap()
    dinv_c = nc.dram_tensor("dinv_c", [D, nt], F32, kind="ExternalInput").ap()
    sdeg_r = nc.dram_tensor("sdeg_r", [1, nt * TILE], BF16, kind="ExternalInput").ap()
    idx_d = nc.dram_tensor("idx", [D, plan.idx_cols], I16, kind="ExternalInput").ap()
    doff_d = nc.dram_tensor("doff", [D, plan.ch_cols], BF16, kind="ExternalInput").ap()
    out_d = nc.dram_tensor("out", [ns, D], F32, kind="ExternalOutput").ap()

    # tile-major storage, split in two tile-halves per layer; each half is
    # dumped contiguously and AllGather'd separately (overlaps compute)
    nth = plan.nth
    hbh = [[nc.dram_tensor(f"h{i}b{h}", [TILE, nth * D], BF16).ap()
            for h in range(2)] for i in range(2)]
    hfh = [[nc.dram_tensor(f"h{i}f{h}", [plan.NH, D], BF16,
                           addr_space="Shared").ap()
            for h in range(2)] for i in range(2)]

    max_C = max(sum(plan.gb_C[g]) for g in range(len(plan.groups)))
    max_icols = max(sum(plan.gb_C[g]) * 8 for g in range(len(plan.groups)))

    with tile.TileContext(nc) as tc:
        with (
            tc.tile_pool(name="const", bufs=1) as cpool,
            tc.tile_pool(name="xstream", bufs=3) as xpool,
            tc.tile_pool(name="stage", bufs=10) as spool,
            tc.tile_pool(name="oh", bufs=3) as ohpool,
            tc.tile_pool(name="aux", bufs=6) as auxpool,
            tc.tile_pool(name="ev", bufs=4) as evpool,
            tc.tile_pool(name="acc", bufs=4, space="PSUM") as accpool,
            tc.tile_pool(name="ptr", bufs=2, space="PSUM") as trpool,
            tc.tile_pool(name="pd", bufs=2, space="PSUM") as pdpool,
        ):
            w_sb = cpool.tile([D, 2 * D], BF16, tag="w")
            nc.sync.dma_start(out=w_sb[:], in_=wts[:])
            co_sb = cpool.tile([D, 2 * D], BF16, tag="co")
            nc.sync.dma_start(out=co_sb[:], in_=consts[:])
            br_sb = cpool.tile([1, 2 * D], BF16, tag="br")
            nc.sync.dma_start(out=br_sb[:], in_=brow[:])
            dv_sb = cpool.tile([D, nt], F32, tag="dv")
            nc.sync.dma_start(out=dv_sb[:], in_=dinv_c[:])
            sd_sb = cpool.tile([1, nt * TILE], BF16, tag="sd")
            nc.sync.dma_start(out=sd_sb[:], in_=sdeg_r[:])
            cn_sb = cpool.tile([1, plan.ncalls], mybir.dt.int32, tag="cn")
            nc.sync.dma_start(out=cn_sb[:], in_=cnts_d[:])
            # local h' tiles, resident for the diagonal (self-loop) matmul
            hl = [cpool.tile([TILE, nt, D], BF16, tag=f"hl{i}", name=f"hl{i}")
                  for i in range(2)]
            gregs = [nc.gpsimd.alloc_register(f"gcnt{i}") for i in range(NQ)]

            W1 = w_sb[:, 0:D]
            W2 = w_sb[:, D:2 * D]
            iota = co_sb[:, 0:D]
            ident = co_sb[:, D:2 * D]

            def tw(t):
                return TILE if t < nt - 1 else plan.last_w

            def dump_half(li, h):
                """hl[li] tile-half h -> hbh[li][h], then AllGather it."""
                nc.sync.dma_start(
                    out=hbh[li][h][:],
                    in_=hl[li][:, h * nth:(h + 1) * nth, :].opt())
                nc.gpsimd.collective_compute(
                    "AllGather", ALU.bypass,
                    replica_groups=[list(range(NCORES))],
                    ins=[hbh[li][h].opt()], outs=[hfh[li][h].opt()])

            # ---- layer-1 dense: h0' = (x @ W1) * dinv ----
            SLAB = 8
            for s in range(0, nt, SLAB):
                sn = min(SLAB, nt - s)
                sw = (sn - 1) * TILE + tw(s + sn - 1)
                xt_t = xpool.tile([D, SLAB * TILE], BF16, tag="xt")
                nc.sync.dma_start(out=xt_t[:, :sw],
                                  in_=xT[:, s * TILE:s * TILE + sw])
                for j in range(sn):
                    t = s + j
                    w = tw(t)
                    pd = pdpool.tile([TILE, D], F32, tag="pd")
                    nc.tensor.matmul(pd[:w, :],
                                     lhsT=xt_t[:, j * TILE:j * TILE + w],
                                     rhs=W1, start=True, stop=True)
                    nc.scalar.activation(hl[0][:w, t, :], pd[:w, :], AF.Copy,
                                         scale=dv_sb[:w, t:t + 1])
                if s <= nth - 1 < s + sn:
                    dump_half(0, 0)   # first tile-half done -> AG overlaps rest
            dump_half(0, 1)

            # ---- sparse layer (templated over layer index) ----
            max_Cgb = max((plan.gb_C[g][b] for g in range(len(plan.groups))
                           for b in range(nblk)), default=1)

            # zero all stage buffers once: trimmed gathers leave untouched
            # slots whose virgin SBUF content may be NaN bit patterns, and
            # the PE turns 0 * NaN into NaN despite the zero one-hot column
            for _i in range(10):
                stg0 = spool.tile([D, max_Cgb, TILE], BF16, tag="stage",
                                  name="stg")
                nc.vector.memset(stg0[:].opt(), 0.0)

            qctr = [0]

            # per (g, b) -> list of (t, kb, c0, cn, call_idx)
            bucket_calls = {}
            for ci, (g, b, t, kb, c0, cn) in enumerate(plan.calls):
                bucket_calls.setdefault((g, b), []).append((t, kb, c0, cn, ci))

            # group index after which hl[li=1] tiles 0..nth-1 are all evicted
            g_half = next(gi for gi, tiles in enumerate(plan.groups)
                          if tiles[-1] >= nth - 1)

            def sparse_layer(li, dump_pending=None):
                tbl = {}       # g -> (idx_sb, do_sb)
                stgs = {}      # (g, b) -> gathered stage tile
                accs_map = {}  # g -> accs

                def load_tables(g):
                    Ctot = sum(plan.gb_C[g])
                    icols = Ctot * 8
                    idx_sb = auxpool.tile([D, max_icols], I16, tag="idx",
                                          name="idx_sb")
                    nc.sync.dma_start(
                        out=idx_sb[:, :icols],
                        in_=idx_d[:, plan.idx_col[g][0]:plan.idx_col[g][0] + icols])
                    do_sb = auxpool.tile([D, max_C], BF16, tag="doff",
                                         name="do_sb")
                    nc.sync.dma_start(
                        out=do_sb[:, :Ctot],
                        in_=doff_d[:, plan.ch_col[g]:plan.ch_col[g] + Ctot])
                    tbl[g] = (idx_sb, do_sb)

                def init_accs(g):
                    accs = {}
                    for t in plan.groups[g]:
                        w = tw(t)
                        accs[t] = accpool.tile([TILE, D], F32, tag="acc", name=f"acc_t{t}")
                        # diagonal (self-loop) term: acc = I @ h'[tile]
                        nc.tensor.matmul(
                            accs[t][:w, :], lhsT=ident[:w, :w],
                            rhs=hl[li][:w, t, :], start=True, stop=False)
                    accs_map[g] = accs

                def do_gather(g, b):
                    idx_sb, _ = tbl[g]
                    Cgb = plan.gb_C[g][b]
                    if Cgb == 0:
                        return
                    ic0 = plan.idx_col[g][b] - plan.idx_col[g][0]
                    stg = spool.tile([D, max_Cgb, TILE], BF16, tag="stage",
                                     name="stg")
                    src_half = hfh[li][b // 2]
                    sbase = (b % 2) * blk
                    for (t, kb, c0, cn, ci) in bucket_calls[(g, b)]:
                        q = qctr[0] % NQ
                        nc.gpsimd.reg_load(gregs[q], cn_sb[0:1, ci:ci + 1])
                        nc.gpsimd.dma_gather(
                            stg[:, kb + c0:kb + c0 + cn, :],
                            src_half[sbase:sbase + blk, :],
                            idx_sb[:, ic0 + (kb + c0) * 8:
                                   ic0 + (kb + c0 + cn) * 8],
                            plan.call_imm[ci],
                            gregs[q],
                            D,
                            queue_num=q,
                        )
                        qctr[0] += 1
                    stgs[(g, b)] = stg

                def do_mm(g, b):
                    Cgb = plan.gb_C[g][b]
                    if Cgb == 0:
                        return
                    _, do_sb = tbl[g]
                    accs = accs_map[g]
                    stg = stgs.pop((g, b))
                    gco = sum(plan.gb_C[g][:b])
                    oh_sb = ohpool.tile([D, max_Cgb, TILE], BF16, tag="oh",
                                        name="oh_sb")
                    nc.vector.scalar_tensor_tensor(
                        out=oh_sb[:, :Cgb, :],
                        in0=do_sb[:, gco:gco + Cgb].unsqueeze(2)
                            .broadcast_to([D, Cgb, TILE]),
                        scalar=1.0,
                        in1=iota.unsqueeze(1).broadcast_to([D, Cgb, TILE]),
                        op0=ALU.mult,
                        op1=ALU.is_equal,
                    )
                    k = 0
                    for (t, nch) in plan.gb_chunks[g][b]:
                        for _ in range(nch):
                            nc.tensor.matmul(
                                accs[t][:], lhsT=oh_sb[:, k, :],
                                rhs=stg[:, k, :],
                                start=False, stop=False)
                            k += 1

                def finish_group(g, li):
                    tiles = plan.groups[g]
                    accs = accs_map[g]
                    for t in tiles:
                        w = tw(t)
                        acc = accs[t]
                        # bias as rank-1: outer(sqrt(deg), b); sdeg rows
                        # beyond the tile width are zero-padded on the host
                        nc.tensor.matmul(
                            acc[:],
                            lhsT=sd_sb[:, t * TILE:(t + 1) * TILE],
                            rhs=br_sb[:, li * D:(li + 1) * D],
                            start=False, stop=True)
                        if li == 0:
                            ev = evpool.tile([TILE, D], BF16, tag="ev")
                            nc.scalar.activation(ev[:w, :], acc[:w, :], AF.Copy,
                                                 scale=dv_sb[:w, t:t + 1])
                            # fused layer-2 dense: h1' = (out1 @ W2) * dinv
                            ptr = trpool.tile([D, TILE], BF16, tag="ptr")
                            nc.tensor.transpose(ptr[:, :w], ev[:w, :],
                                                ident[:w, :w])
                            trs = evpool.tile([D, TILE], BF16, tag="trs")
                            nc.vector.tensor_copy(trs[:, :w], ptr[:, :w])
                            pd = pdpool.tile([TILE, D], F32, tag="pd")
                            nc.tensor.matmul(pd[:w, :], lhsT=trs[:, :w], rhs=W2,
                                             start=True, stop=True)
                            nc.scalar.activation(hl[1][:w, t, :], pd[:w, :],
                                                 AF.Copy,
                                                 scale=dv_sb[:w, t:t + 1])
                        else:
                            evf = evpool.tile([TILE, D], F32, tag="evf")
                            nc.scalar.activation(evf[:w, :], acc[:w, :], AF.Copy,
                                                 scale=dv_sb[:w, t:t + 1])
                            nc.sync.dma_start(
                                out=out_d[t * TILE:t * TILE + w, :],
                                in_=evf[:w, :])

                    del accs_map[g]
                    if li == 0 and g == g_half:
                        # first tile-half of h1' is complete: AG it now so it
                        # overlaps the rest of sparse layer 1
                        dump_half(1, 0)

                # first K groups: gather blocks 0-1 (first-half AG) before any
                # block 2-3 gather, so the second-half AG completes under work
                ngroups = len(plan.groups)
                K = min(4, ngroups)
                for g in range(K):
                    load_tables(g)
                    do_gather(g, 0)
                    do_gather(g, 1)
                if dump_pending is not None:
                    # second-half dump+AG of the PREVIOUS layer's output is
                    # emitted here so its input-dependency stall does not
                    # block phase-A's sync loads / gpsimd gathers
                    dump_half(*dump_pending)
                for g in range(K):
                    init_accs(g)
                    do_mm(g, 0)
                    do_mm(g, 1)
                    do_gather(g, 2)
                    do_mm(g, 2)
                    do_gather(g, 3)
                    do_mm(g, 3)
                    finish_group(g, li)
                for g in range(K, ngroups):
                    load_tables(g)
                    init_accs(g)
                    for b in range(nblk):
                        do_gather(g, b)
                        do_mm(g, b)
                    finish_group(g, li)

            sparse_layer(0)
            sparse_layer(1, dump_pending=(1, 1))

    nc.compile()
    return nc


def _install_ntff_hook():
    """antenv.axon_hooks is absent in this image; synthesize it and register
    the ctypes NTFF profile hook from the boot module."""
    import types
    if "antenv.axon_hooks" in sys.modules:
        return
    try:
        from trn_agent_boot.trn_boot import _ntff_profile_via_ctypes
        hook = _ntff_profile_via_ctypes("/opt/axon/libaxon_pjrt.so")
    except Exception as e:
        print(f"[kernel] ntff hook unavailable: {e}", flush=True)
        hook = None
    mod = types.ModuleType("antenv.axon_hooks")
    mod._hook = hook
    mod.set_axon_ntff_profile_hook = lambda h: setattr(mod, "_hook", h)
    mod.get_axon_ntff_profile_hook = lambda: mod._hook
    sys.modules["antenv.axon_hooks"] = mod
    import antenv
    antenv.axon_hooks = mod


def _run(plan, x, W1, b1, W2, b2, trace=False, stage="full"):
    import time
    if trace:
        _install_ntff_hook()
    t0 = time.time()
    nc = _build(plan)
    t1 = time.time()
    if os.environ.get("GCN_VERBOSE"):
        print(f"[kernel] build+compile: {t1 - t0:.1f}s", flush=True)
    ns, nt = plan.ns, plan.nt
    iota_t = np.tile(np.arange(TILE, dtype=np.float32), (TILE, 1))
    ident_t = np.eye(TILE, dtype=np.float32)
    consts = np.concatenate([iota_t, ident_t], axis=1).astype(NPBF)
    wts = np.concatenate([W1.astype(np.float32), W2.astype(np.float32)],
                         axis=1).astype(NPBF)
    brow = np.concatenate([b1.astype(np.float32), b2.astype(np.float32)]
                          ).reshape(1, 2 * D).astype(NPBF)

    in_maps = []
    for c in range(NCORES):
        lo, hi = c * ns, (c + 1) * ns
        dv = plan.dinv[lo:hi]
        # column t of dcol holds dinv[lo + t*128 : lo + (t+1)*128] (pad 1.0)
        dcol = np.ones((nt, TILE), dtype=np.float32)
        dcol.reshape(-1)[:ns] = dv
        dcol = np.ascontiguousarray(dcol.T)
        sdr = np.zeros((1, nt * TILE), dtype=np.float32)
        sdr[0, :ns] = plan.sdeg[lo:hi]
        idx, doff, cnts = plan.core_inputs(c)
        in_maps.append({
            "xT": np.ascontiguousarray(x[lo:hi].astype(np.float32).T).astype(NPBF),
            "wts": wts, "consts": consts, "brow": brow,
            "dinv_c": dcol, "sdeg_r": sdr.astype(NPBF),
            "idx": idx, "doff": doff, "cnts": cnts,
        })
    t2 = time.time()
    res = run_bass_kernel_spmd(nc, in_maps, core_ids=list(range(NCORES)),
                               trace=trace)
    if os.environ.get("GCN_VERBOSE"):
        print(f"[kernel] prep inputs: {t2 - t1:.1f}s, run: {time.time() - t2:.1f}s",
              flush=True)
    out = np.concatenate([res.results[c]["out"] for c in range(NCORES)], axis=0)
    return out, res


def kernel(x, edge_index, W1, b1, W2, b2):
    plan = Plan(x.shape[0], np.asarray(edge_index))
    out, _ = _run(plan, np.asarray(x), np.asarray(W1), np.asarray(b1),
                  np.asarray(W2), np.asarray(b2))
    return out


# revision 62
# speedup vs baseline: 1.0094x; 1.0076x over previous
"""Trainium2 Bass kernel: 2-layer GCN (PyG-style GCNConv x2) on 8 NeuronCores.

Strategy:
  - Nodes sharded contiguously across 8 cores (12500 rows each).
  - Per layer: dense h' = (x @ W) * dinv[row] computed on the owning core
    (bf16) into a resident SBUF tile (hl), dumped to DRAM in one contiguous
    DMA (tile-major layout), AllGather'd to every core (25.6MB replica),
    then per-core sparse aggregation over its in-edges:
      gather h'[src] rows via dma_gather (int16 idx into 4 blocks of 25088
      storage positions; 4 SWDGE queues round-robin so Q7 descriptor
      emission runs on all four core pairs concurrently; per-core exact
      counts via num_idxs_reg + trailing -1 idx so padding is never
      gathered; per-bucket idx sorted ascending for HBM row locality),
      scatter-add via one-hot matmul into PSUM per 128-dst tile,
      self-loops as a diagonal identity matmul from the resident hl tile,
      bias added as rank-1 matmul outer(sqrt(deg), b),
      eviction scaled by dinv[dst] on the scalar engine (layer 1 fuses the
      layer-2 dense transform behind a PE transpose).
  - The per-edge norm dinv[src]*dinv[dst] is folded into the two node-level
    scalings, so no per-edge multiply exists anywhere.
  - All matmul/gather traffic in bf16 (tolerance 2e-2 >> bf16 error);
    PSUM accumulation fp32; final output fp32.
"""

import os
import sys

for _p in ("/opt/trn_rl_repo",):
    if _p not in sys.path:
        sys.path.append(_p)

import numpy as np
import ml_dtypes

import concourse.bacc as bacc
import concourse.bass as bass
import concourse.mybir as mybir
import concourse.tile as tile
from concourse.bass_utils import run_bass_kernel_spmd

F32 = mybir.dt.float32
BF16 = mybir.dt.bfloat16
I16 = mybir.dt.int16
AF = mybir.ActivationFunctionType
ALU = mybir.AluOpType
NPBF = ml_dtypes.bfloat16

N_NODES = 100000
D = 128
NCORES = 8
TILE = 128
NQ = 4  # SWDGE queues round-robin for dma_gather descriptor emission


def _ceil_div(a, b):
    return (a + b - 1) // b


class Plan:
    """Core-uniform structure tables derived from the edge index."""

    def __init__(self, n_nodes, edge_index, group_tiles=4):
        self.n = n_nodes
        self.ns = n_nodes // NCORES            # nodes per core
        self.nt = _ceil_div(self.ns, TILE)     # dst tiles per core
        self.last_w = self.ns - (self.nt - 1) * TILE
        self.G = group_tiles
        # h' is stored tile-major per core and AllGather'd in 4 tile-quarters
        # so each AG fires early and overlaps surrounding work.  Gather block
        # b == quarter b; storage pos of node (c, t, rr) within quarter q:
        # qbase[q] + (c*128+rr)*qt[q] + (t - qstart[q])
        base, rem = self.nt // 4, self.nt % 4
        self.qt = [base + (1 if q < rem else 0) for q in range(4)]
        self.qstart = [sum(self.qt[:q]) for q in range(4)]
        self.qsize = [NCORES * TILE * t for t in self.qt]
        self.qbase = [sum(self.qsize[:q]) for q in range(4)]
        self.nblk = 4                          # src blocks (int16 idx limit)
        assert max(self.qsize) <= 32768        # int16 idx range

        # degree includes self-loops (PyG GCNConv semantics) but the self
        # edges are NOT gathered: handled as a diagonal identity matmul.
        deg = np.bincount(edge_index[1], minlength=n_nodes).astype(np.float32)
        deg += 1.0
        self.dinv = deg ** -0.5
        self.sdeg = np.sqrt(deg)
        src = np.asarray(edge_index[0])
        dst = np.asarray(edge_index[1])

        core = dst // self.ns
        tloc = (dst % self.ns) // TILE
        sc = src // self.ns
        sr = src % self.ns
        st = sr // TILE
        qof = np.zeros(self.nt, np.int64)
        for q in range(4):
            qof[self.qstart[q]:self.qstart[q] + self.qt[q]] = q
        blk = qof[st]
        qb = np.asarray(self.qbase)[blk]
        qtl = np.asarray(self.qt)[blk]
        qst = np.asarray(self.qstart)[blk]
        pos = qb + (sc * TILE + (sr % TILE)) * qtl + (st - qst)
        key = (core * self.nt + tloc) * self.nblk + blk
        # ascending storage pos within each (core,tile,blk) bucket: the
        # per-bucket gather descriptors then sweep HBM monotonically
        order = np.lexsort((pos, key))
        self.pos_s = pos[order]
        self.doff_s = ((dst % self.ns) % TILE)[order]
        counts = np.bincount(key, minlength=NCORES * self.nt * self.nblk)
        self.counts = counts.reshape(NCORES, self.nt, self.nblk)
        # segment start offsets into src_s per (core, tile, blk)
        self.seg_off = np.zeros(NCORES * self.nt * self.nblk + 1, dtype=np.int64)
        np.cumsum(counts, out=self.seg_off[1:])

        # chunks per (tile, blk): shared across cores
        self.CT = _ceil_div(self.counts, TILE).max(axis=0)  # [nt, nblk]

        # tile groups
        self.groups = [list(range(g, min(g + self.G, self.nt)))
                       for g in range(0, self.nt, self.G)]

        # per (group, blk): chunk count and the (tile, n_chunks) layout
        self.gb_chunks = []   # [g][b] -> list of (tile, CT[t][b])
        self.gb_C = []        # [g][b] -> total chunks
        for tiles in self.groups:
            row_l, row_c = [], []
            for b in range(self.nblk):
                lay = [(t, int(self.CT[t, b])) for t in tiles if self.CT[t, b] > 0]
                row_l.append(lay)
                row_c.append(sum(c for _, c in lay))
            self.gb_chunks.append(row_l)
            self.gb_C.append(row_c)

        # dma_gather call schedule: one call never spans a (tile, blk)
        # segment so per-core trailing -1 idx trimming (num_idxs_reg) works
        self.GMAX = 8
        self.calls = []   # (g, b, t, kb: chunk col in bucket, c0: within seg, cn)
        for g in range(len(self.groups)):
            for b in range(self.nblk):
                kb = 0
                for (t, nch) in self.gb_chunks[g][b]:
                    for c0 in range(0, nch, self.GMAX):
                        cn = min(self.GMAX, nch - c0)
                        self.calls.append((g, b, t, kb, c0, cn))
                    kb += nch
        self.ncalls = len(self.calls)
        # tight per-call num_idxs immediate: the max real count over cores,
        # rounded to 16 (the ucode's idx-unpack cost scales with it)
        self.call_imm = []
        for (g, b, t, kb, c0, cn) in self.calls:
            mx = int(self.counts[:, t, b].max()) - c0 * TILE
            mx = min(max(mx, 1), cn * TILE)
            imm = min(((mx + 15) // 16) * 16, cn * TILE)
            assert imm > (cn - 1) * TILE, (imm, cn)
            self.call_imm.append(imm)

        # column offsets in the concatenated idx / dstoff DRAM buffers
        self.idx_col = []     # [g][b] -> start col in idx buffer (int16, /16 wrap)
        self.ch_col = []      # [g] -> start chunk col in dstoff buffer
        ic = 0
        cc = 0
        for g in range(len(self.groups)):
            self.ch_col.append(cc)
            row = []
            for b in range(self.nblk):
                row.append(ic)
                ic += self.gb_C[g][b] * (TILE // 16)
                cc += self.gb_C[g][b]
            self.idx_col.append(row)
        self.idx_cols = ic
        self.ch_cols = cc

    def core_inputs(self, c):
        """Build idx (int16, -1 pad), dstoff (bf16) and per-call counts."""
        idx = np.full((16, self.idx_cols), -1, dtype=np.int16)
        doff = np.full((128, self.ch_cols), -1.0, dtype=np.float32)
        for g, tiles in enumerate(self.groups):
            ch = self.ch_col[g]
            for b in range(self.nblk):
                icol = self.idx_col[g][b]
                for (t, nch) in self.gb_chunks[g][b]:
                    cnt = int(self.counts[c, t, b])
                    o = self.seg_off[(c * self.nt + t) * self.nblk + b]
                    nslots = nch * TILE
                    a = np.full(nslots, -1, dtype=np.int16)
                    a[:cnt] = (self.pos_s[o:o + cnt] - self.qbase[b]).astype(np.int16)
                    idx[:, icol:icol + nch * 8] = a.reshape(nch * 8, 16).T
                    dv = np.full(nslots, -1.0, dtype=np.float32)
                    dv[:cnt] = self.doff_s[o:o + cnt].astype(np.float32)
                    doff[:, ch:ch + nch] = dv.reshape(nch, 128).T
                    icol += nch * 8
                    ch += nch
        idx_full = np.tile(idx, (8, 1))
        cnts = np.zeros((1, self.ncalls), dtype=np.int32)
        for i, (g, b, t, kb, c0, cn) in enumerate(self.calls):
            cnt = int(self.counts[c, t, b])
            cnts[0, i] = min(max(cnt - c0 * TILE, 0), cn * TILE)
        return idx_full, doff.astype(NPBF), cnts


def _build(plan):
    """Build the SPMD bass program (shared by all 8 cores)."""
    n, ns, nt, nblk = plan.n, plan.ns, plan.nt, plan.nblk
    nc = bacc.Bacc("TRN2", target_bir_lowering=False, debug=False,
                   num_devices=NCORES, num_swdge_queues=NQ)

    I32 = mybir.dt.int32
    xT = nc.dram_tensor("xT", [D, ns], BF16, kind="ExternalInput").ap()
    cnts_d = nc.dram_tensor("cnts", [1, plan.ncalls], I32,
                            kind="ExternalInput").ap()
    wts = nc.dram_tensor("wts", [D, 2 * D], BF16, kind="ExternalInput").ap()
    consts = nc.dram_tensor("consts", [D, 2 * D], BF16, kind="ExternalInput").ap()
    brow = nc.dram_tensor("brow", [1, 2 * D], BF16, kind="ExternalInput").ap()
    dinv_c = nc.dram_tensor("dinv_c", [D, nt], F32, kind="ExternalInput").ap()
    sdeg_r = nc.dram_tensor("sdeg_r", [1, nt * TILE], BF16, kind="ExternalInput").ap()
    idx_d = nc.dram_tensor("idx", [D, plan.idx_cols], I16, kind="ExternalInput").ap()
    doff_d = nc.dram_tensor("doff", [D, plan.ch_cols], BF16, kind="ExternalInput").ap()
    out_d = nc.dram_tensor("out", [ns, D], F32, kind="ExternalOutput").ap()

    # tile-major storage, split in 4 tile-quarters per layer; each quarter
    # is dumped contiguously and AllGather'd separately (overlaps compute)
    qs, qt_, qsz = plan.qstart, plan.qt, plan.qsize
    hbq = [[nc.dram_tensor(f"h{i}b{q}", [TILE, qt_[q] * D], BF16).ap()
            for q in range(4)] for i in range(2)]
    hfq = [[nc.dram_tensor(f"h{i}f{q}", [qsz[q], D], BF16,
                           addr_space="Shared").ap()
            for q in range(4)] for i in range(2)]

    max_C = max(sum(plan.gb_C[g]) for g in range(len(plan.groups)))
    max_icols = max(sum(plan.gb_C[g]) * 8 for g in range(len(plan.groups)))

    with tile.TileContext(nc) as tc:
        with (
            tc.tile_pool(name="const", bufs=1) as cpool,
            tc.tile_pool(name="xstream", bufs=3) as xpool,
            tc.tile_pool(name="stage", bufs=10) as spool,
            tc.tile_pool(name="oh", bufs=3) as ohpool,
            tc.tile_pool(name="aux", bufs=6) as auxpool,
            tc.tile_pool(name="ev", bufs=4) as evpool,
            tc.tile_pool(name="acc", bufs=4, space="PSUM") as accpool,
            tc.tile_pool(name="ptr", bufs=2, space="PSUM") as trpool,
            tc.tile_pool(name="pd", bufs=2, space="PSUM") as pdpool,
        ):
            w_sb = cpool.tile([D, 2 * D], BF16, tag="w")
            nc.sync.dma_start(out=w_sb[:], in_=wts[:])
            co_sb = cpool.tile([D, 2 * D], BF16, tag="co")
            nc.sync.dma_start(out=co_sb[:], in_=consts[:])
            br_sb = cpool.tile([1, 2 * D], BF16, tag="br")
            nc.sync.dma_start(out=br_sb[:], in_=brow[:])
            dv_sb = cpool.tile([D, nt], F32, tag="dv")
            nc.sync.dma_start(out=dv_sb[:], in_=dinv_c[:])
            sd_sb = cpool.tile([1, nt * TILE], BF16, tag="sd")
            nc.sync.dma_start(out=sd_sb[:], in_=sdeg_r[:])
            cn_sb = cpool.tile([1, plan.ncalls], mybir.dt.int32, tag="cn")
            nc.sync.dma_start(out=cn_sb[:], in_=cnts_d[:])
            # local h' tiles, resident for the diagonal (self-loop) matmul
            hl = [cpool.tile([TILE, nt, D], BF16, tag=f"hl{i}", name=f"hl{i}")
                  for i in range(2)]
            gregs = [nc.gpsimd.alloc_register(f"gcnt{i}") for i in range(NQ)]

            W1 = w_sb[:, 0:D]
            W2 = w_sb[:, D:2 * D]
            iota = co_sb[:, 0:D]
            ident = co_sb[:, D:2 * D]

            def tw(t):
                return TILE if t < nt - 1 else plan.last_w

            def dump_quarter(li, q):
                """hl[li] tile-quarter q -> hbq[li][q], then AllGather it."""
                nc.sync.dma_start(
                    out=hbq[li][q][:],
                    in_=hl[li][:, qs[q]:qs[q] + qt_[q], :].opt())
                nc.gpsimd.collective_compute(
                    "AllGather", ALU.bypass,
                    replica_groups=[list(range(NCORES))],
                    ins=[hbq[li][q].opt()], outs=[hfq[li][q].opt()])

            # ---- layer-1 dense: h0' = (x @ W1) * dinv ----
            SLAB = 8
            for s in range(0, nt, SLAB):
                sn = min(SLAB, nt - s)
                sw = (sn - 1) * TILE + tw(s + sn - 1)
                xt_t = xpool.tile([D, SLAB * TILE], BF16, tag="xt")
                nc.sync.dma_start(out=xt_t[:, :sw],
                                  in_=xT[:, s * TILE:s * TILE + sw])
                for j in range(sn):
                    t = s + j
                    w = tw(t)
                    pd = pdpool.tile([TILE, D], F32, tag="pd")
                    nc.tensor.matmul(pd[:w, :],
                                     lhsT=xt_t[:, j * TILE:j * TILE + w],
                                     rhs=W1, start=True, stop=True)
                    nc.scalar.activation(hl[0][:w, t, :], pd[:w, :], AF.Copy,
                                         scale=dv_sb[:w, t:t + 1])
                for q in range(3):
                    if s <= qs[q] + qt_[q] - 1 < s + sn:
                        dump_quarter(0, q)  # quarter done -> AG overlaps rest
            dump_quarter(0, 3)

            # ---- sparse layer (templated over layer index) ----
            max_Cgb = max((plan.gb_C[g][b] for g in range(len(plan.groups))
                           for b in range(nblk)), default=1)

            # zero all stage buffers once: trimmed gathers leave untouched
            # slots whose virgin SBUF content may be NaN bit patterns, and
            # the PE turns 0 * NaN into NaN despite the zero one-hot column
            for _i in range(10):
                stg0 = spool.tile([D, max_Cgb, TILE], BF16, tag="stage",
                                  name="stg")
                nc.vector.memset(stg0[:].opt(), 0.0)

            qctr = [0]

            # per (g, b) -> list of (t, kb, c0, cn, call_idx)
            bucket_calls = {}
            for ci, (g, b, t, kb, c0, cn) in enumerate(plan.calls):
                bucket_calls.setdefault((g, b), []).append((t, kb, c0, cn, ci))

            # group index after which hl[li=1] quarter q is fully evicted
            qhooks = {}
            for q in range(3):
                qlast = qs[q] + qt_[q] - 1
                gi = next(gi for gi, tiles in enumerate(plan.groups)
                          if tiles[-1] >= qlast)
                qhooks[gi] = q

            def sparse_layer(li):
                tbl = {}       # g -> (idx_sb, do_sb)
                stgs = {}      # (g, b) -> gathered stage tile
                accs_map = {}  # g -> accs

                def load_tables(g):
                    Ctot = sum(plan.gb_C[g])
                    icols = Ctot * 8
                    idx_sb = auxpool.tile([D, max_icols], I16, tag="idx",
                                          name="idx_sb")
                    nc.sync.dma_start(
                        out=idx_sb[:, :icols],
                        in_=idx_d[:, plan.idx_col[g][0]:plan.idx_col[g][0] + icols])
                    do_sb = auxpool.tile([D, max_C], BF16, tag="doff",
                                         name="do_sb")
                    nc.sync.dma_start(
                        out=do_sb[:, :Ctot],
                        in_=doff_d[:, plan.ch_col[g]:plan.ch_col[g] + Ctot])
                    tbl[g] = (idx_sb, do_sb)

                def init_accs(g):
                    accs = {}
                    for t in plan.groups[g]:
                        w = tw(t)
                        accs[t] = accpool.tile([TILE, D], F32, tag="acc", name=f"acc_t{t}")
                        # diagonal (self-loop) term: acc = I @ h'[tile]
                        nc.tensor.matmul(
                            accs[t][:w, :], lhsT=ident[:w, :w],
                            rhs=hl[li][:w, t, :], start=True, stop=False)
                    accs_map[g] = accs

                def do_gather(g, b):
                    idx_sb, _ = tbl[g]
                    Cgb = plan.gb_C[g][b]
                    if Cgb == 0:
                        return
                    ic0 = plan.idx_col[g][b] - plan.idx_col[g][0]
                    stg = spool.tile([D, max_Cgb, TILE], BF16, tag="stage",
                                     name="stg")
                    src_q = hfq[li][b]
                    for (t, kb, c0, cn, ci) in bucket_calls[(g, b)]:
                        q = qctr[0] % NQ
                        nc.gpsimd.reg_load(gregs[q], cn_sb[0:1, ci:ci + 1])
                        nc.gpsimd.dma_gather(
                            stg[:, kb + c0:kb + c0 + cn, :],
                            src_q[0:qsz[b], :],
                            idx_sb[:, ic0 + (kb + c0) * 8:
                                   ic0 + (kb + c0 + cn) * 8],
                            plan.call_imm[ci],
                            gregs[q],
                            D,
                            queue_num=q,
                        )
                        qctr[0] += 1
                    stgs[(g, b)] = stg

                def do_mm(g, b):
                    Cgb = plan.gb_C[g][b]
                    if Cgb == 0:
                        return
                    _, do_sb = tbl[g]
                    accs = accs_map[g]
                    stg = stgs.pop((g, b))
                    gco = sum(plan.gb_C[g][:b])
                    oh_sb = ohpool.tile([D, max_Cgb, TILE], BF16, tag="oh",
                                        name="oh_sb")
                    nc.vector.scalar_tensor_tensor(
                        out=oh_sb[:, :Cgb, :],
                        in0=do_sb[:, gco:gco + Cgb].unsqueeze(2)
                            .broadcast_to([D, Cgb, TILE]),
                        scalar=1.0,
                        in1=iota.unsqueeze(1).broadcast_to([D, Cgb, TILE]),
                        op0=ALU.mult,
                        op1=ALU.is_equal,
                    )
                    k = 0
                    for (t, nch) in plan.gb_chunks[g][b]:
                        for _ in range(nch):
                            nc.tensor.matmul(
                                accs[t][:], lhsT=oh_sb[:, k, :],
                                rhs=stg[:, k, :],
                                start=False, stop=False)
                            k += 1

                def finish_group(g, li):
                    tiles = plan.groups[g]
                    accs = accs_map[g]
                    for t in tiles:
                        w = tw(t)
                        acc = accs[t]
                        # bias as rank-1: outer(sqrt(deg), b); sdeg rows
                        # beyond the tile width are zero-padded on the host
                        nc.tensor.matmul(
                            acc[:],
                            lhsT=sd_sb[:, t * TILE:(t + 1) * TILE],
                            rhs=br_sb[:, li * D:(li + 1) * D],
                            start=False, stop=True)
                        if li == 0:
                            ev = evpool.tile([TILE, D], BF16, tag="ev")
                            nc.scalar.activation(ev[:w, :], acc[:w, :], AF.Copy,
                                                 scale=dv_sb[:w, t:t + 1])
                            # fused layer-2 dense: h1' = (out1 @ W2) * dinv
                            ptr = trpool.tile([D, TILE], BF16, tag="ptr")
                            nc.tensor.transpose(ptr[:, :w], ev[:w, :],
                                                ident[:w, :w])
                            trs = evpool.tile([D, TILE], BF16, tag="trs")
                            nc.vector.tensor_copy(trs[:, :w], ptr[:, :w])
                            pd = pdpool.tile([TILE, D], F32, tag="pd")
                            nc.tensor.matmul(pd[:w, :], lhsT=trs[:, :w], rhs=W2,
                                             start=True, stop=True)
                            nc.scalar.activation(hl[1][:w, t, :], pd[:w, :],
                                                 AF.Copy,
                                                 scale=dv_sb[:w, t:t + 1])
                        else:
                            evf = evpool.tile([TILE, D], F32, tag="evf")
                            nc.scalar.activation(evf[:w, :], acc[:w, :], AF.Copy,
                                                 scale=dv_sb[:w, t:t + 1])
                            nc.sync.dma_start(
                                out=out_d[t * TILE:t * TILE + w, :],
                                in_=evf[:w, :])

                    del accs_map[g]
                    if li == 0 and g in qhooks:
                        # a tile-quarter of h1' is complete: AG it now so it
                        # overlaps the rest of sparse layer 1
                        dump_quarter(1, qhooks[g])

                # first K groups: gather blocks 0-1 (first-half AG) before any
                # block 2-3 gather, so the second-half AG completes under work
                ngroups = len(plan.groups)
                K = min(4, ngroups)
                for g in range(K):
                    load_tables(g)
                    do_gather(g, 0)
                    do_gather(g, 1)
                for g in range(K):
                    init_accs(g)
                    do_mm(g, 0)
                    do_mm(g, 1)
                    do_gather(g, 2)
                    do_mm(g, 2)
                    do_gather(g, 3)
                    do_mm(g, 3)
                    finish_group(g, li)
                for g in range(K, ngroups):
                    load_tables(g)
                    init_accs(g)
                    for b in range(nblk):
                        do_gather(g, b)
                        do_mm(g, b)
                    finish_group(g, li)

            sparse_layer(0)
            dump_quarter(1, 3)
            sparse_layer(1)

    nc.compile()
    return nc


def _install_ntff_hook():
    """antenv.axon_hooks is absent in this image; synthesize it and register
    the ctypes NTFF profile hook from the boot module."""
    import types
    if "antenv.axon_hooks" in sys.modules:
        return
    try:
        from trn_agent_boot.trn_boot import _ntff_profile_via_ctypes
        hook = _ntff_profile_via_ctypes("/opt/axon/libaxon_pjrt.so")
    except Exception as e:
        print(f"[kernel] ntff hook unavailable: {e}", flush=True)
        hook = None
    mod = types.ModuleType("antenv.axon_hooks")
    mod._hook = hook
    mod.set_axon_ntff_profile_hook = lambda h: setattr(mod, "_hook", h)
    mod.get_axon_ntff_profile_hook = lambda: mod._hook
    sys.modules["antenv.axon_hooks"] = mod
    import antenv
    antenv.axon_hooks = mod


def _run(plan, x, W1, b1, W2, b2, trace=False, stage="full"):
    import time
    if trace:
        _install_ntff_hook()
    t0 = time.time()
    nc = _build(plan)
    t1 = time.time()
    if os.environ.get("GCN_VERBOSE"):
        print(f"[kernel] build+compile: {t1 - t0:.1f}s", flush=True)
    ns, nt = plan.ns, plan.nt
    iota_t = np.tile(np.arange(TILE, dtype=np.float32), (TILE, 1))
    ident_t = np.eye(TILE, dtype=np.float32)
    consts = np.concatenate([iota_t, ident_t], axis=1).astype(NPBF)
    wts = np.concatenate([W1.astype(np.float32), W2.astype(np.float32)],
                         axis=1).astype(NPBF)
    brow = np.concatenate([b1.astype(np.float32), b2.astype(np.float32)]
                          ).reshape(1, 2 * D).astype(NPBF)

    in_maps = []
    for c in range(NCORES):
        lo, hi = c * ns, (c + 1) * ns
        dv = plan.dinv[lo:hi]
        # column t of dcol holds dinv[lo + t*128 : lo + (t+1)*128] (pad 1.0)
        dcol = np.ones((nt, TILE), dtype=np.float32)
        dcol.reshape(-1)[:ns] = dv
        dcol = np.ascontiguousarray(dcol.T)
        sdr = np.zeros((1, nt * TILE), dtype=np.float32)
        sdr[0, :ns] = plan.sdeg[lo:hi]
        idx, doff, cnts = plan.core_inputs(c)
        in_maps.append({
            "xT": np.ascontiguousarray(x[lo:hi].astype(np.float32).T).astype(NPBF),
            "wts": wts, "consts": consts, "brow": brow,
            "dinv_c": dcol, "sdeg_r": sdr.astype(NPBF),
            "idx": idx, "doff": doff, "cnts": cnts,
        })
    t2 = time.time()
    res = run_bass_kernel_spmd(nc, in_maps, core_ids=list(range(NCORES)),
                               trace=trace)
    if os.environ.get("GCN_VERBOSE"):
        print(f"[kernel] prep inputs: {t2 - t1:.1f}s, run: {time.time() - t2:.1f}s",
              flush=True)
    out = np.concatenate([res.results[c]["out"] for c in range(NCORES)], axis=0)
    return out, res


def kernel(x, edge_index, W1, b1, W2, b2):
    plan = Plan(x.shape[0], np.asarray(edge_index))
    out, _ = _run(plan, np.asarray(x), np.asarray(W1), np.asarray(b1),
                  np.asarray(W2), np.asarray(b2))
    return out


# revision 64
# speedup vs baseline: 1.0100x; 1.0005x over previous
"""Trainium2 Bass kernel: 2-layer GCN (PyG-style GCNConv x2) on 8 NeuronCores.

Strategy:
  - Nodes sharded contiguously across 8 cores (12500 rows each).
  - Per layer: dense h' = (x @ W) * dinv[row] computed on the owning core
    (bf16) into a resident SBUF tile (hl), dumped to DRAM in one contiguous
    DMA (tile-major layout), AllGather'd to every core (25.6MB replica),
    then per-core sparse aggregation over its in-edges:
      gather h'[src] rows via dma_gather (int16 idx into 4 blocks of 25088
      storage positions; 4 SWDGE queues round-robin so Q7 descriptor
      emission runs on all four core pairs concurrently; per-core exact
      counts via num_idxs_reg + trailing -1 idx so padding is never
      gathered; per-bucket idx sorted ascending for HBM row locality),
      scatter-add via one-hot matmul into PSUM per 128-dst tile,
      self-loops as a diagonal identity matmul from the resident hl tile,
      bias added as rank-1 matmul outer(sqrt(deg), b),
      eviction scaled by dinv[dst] on the scalar engine (layer 1 fuses the
      layer-2 dense transform behind a PE transpose).
  - The per-edge norm dinv[src]*dinv[dst] is folded into the two node-level
    scalings, so no per-edge multiply exists anywhere.
  - All matmul/gather traffic in bf16 (tolerance 2e-2 >> bf16 error);
    PSUM accumulation fp32; final output fp32.
"""

import os
import sys

for _p in ("/opt/trn_rl_repo",):
    if _p not in sys.path:
        sys.path.append(_p)

import numpy as np
import ml_dtypes

import concourse.bacc as bacc
import concourse.bass as bass
import concourse.mybir as mybir
import concourse.tile as tile
from concourse.bass_utils import run_bass_kernel_spmd

F32 = mybir.dt.float32
BF16 = mybir.dt.bfloat16
I16 = mybir.dt.int16
AF = mybir.ActivationFunctionType
ALU = mybir.AluOpType
NPBF = ml_dtypes.bfloat16

N_NODES = 100000
D = 128
NCORES = 8
TILE = 128
NQ = 4  # SWDGE queues round-robin for dma_gather descriptor emission


def _ceil_div(a, b):
    return (a + b - 1) // b


class Plan:
    """Core-uniform structure tables derived from the edge index."""

    def __init__(self, n_nodes, edge_index, group_tiles=4):
        self.n = n_nodes
        self.ns = n_nodes // NCORES            # nodes per core
        self.nt = _ceil_div(self.ns, TILE)     # dst tiles per core
        self.last_w = self.ns - (self.nt - 1) * TILE
        self.G = group_tiles
        # h' is stored tile-major per core and AllGather'd in two halves
        # (tiles < nth, tiles >= nth) so each AG overlaps surrounding work.
        # storage pos of node (c, t, rr): half*NH + (c*128+rr)*nth + (t%nth)
        self.nth = self.nt // 2                # tiles per half (49)
        self.NH = NCORES * TILE * self.nth     # positions per half
        self.npos = 2 * self.NH
        self.nblk = 4                          # src blocks (int16 idx limit)
        self.blk = self.npos // self.nblk      # 2 blocks per half

        # degree includes self-loops (PyG GCNConv semantics) but the self
        # edges are NOT gathered: handled as a diagonal identity matmul.
        deg = np.bincount(edge_index[1], minlength=n_nodes).astype(np.float32)
        deg += 1.0
        self.dinv = deg ** -0.5
        self.sdeg = np.sqrt(deg)
        src = np.asarray(edge_index[0])
        dst = np.asarray(edge_index[1])

        core = dst // self.ns
        tloc = (dst % self.ns) // TILE
        sc = src // self.ns
        sr = src % self.ns
        st = sr // TILE
        half = (st >= self.nth).astype(np.int64)
        pos = (half * self.NH + (sc * TILE + (sr % TILE)) * self.nth
               + (st - half * self.nth))
        blk = pos // self.blk
        key = (core * self.nt + tloc) * self.nblk + blk
        # ascending storage pos within each (core,tile,blk) bucket: the
        # per-bucket gather descriptors then sweep HBM monotonically
        order = np.lexsort((pos, key))
        self.pos_s = pos[order]
        self.doff_s = ((dst % self.ns) % TILE)[order]
        counts = np.bincount(key, minlength=NCORES * self.nt * self.nblk)
        self.counts = counts.reshape(NCORES, self.nt, self.nblk)
        # segment start offsets into src_s per (core, tile, blk)
        self.seg_off = np.zeros(NCORES * self.nt * self.nblk + 1, dtype=np.int64)
        np.cumsum(counts, out=self.seg_off[1:])

        # chunks per (tile, blk): shared across cores
        self.CT = _ceil_div(self.counts, TILE).max(axis=0)  # [nt, nblk]

        # tile groups
        self.groups = [list(range(g, min(g + self.G, self.nt)))
                       for g in range(0, self.nt, self.G)]

        # per (group, blk): chunk count and the (tile, n_chunks) layout
        self.gb_chunks = []   # [g][b] -> list of (tile, CT[t][b])
        self.gb_C = []        # [g][b] -> total chunks
        for tiles in self.groups:
            row_l, row_c = [], []
            for b in range(self.nblk):
                lay = [(t, int(self.CT[t, b])) for t in tiles if self.CT[t, b] > 0]
                row_l.append(lay)
                row_c.append(sum(c for _, c in lay))
            self.gb_chunks.append(row_l)
            self.gb_C.append(row_c)

        # dma_gather call schedule: one call never spans a (tile, blk)
        # segment so per-core trailing -1 idx trimming (num_idxs_reg) works
        self.GMAX = 8
        self.calls = []   # (g, b, t, kb: chunk col in bucket, c0: within seg, cn)
        for g in range(len(self.groups)):
            for b in range(self.nblk):
                kb = 0
                for (t, nch) in self.gb_chunks[g][b]:
                    for c0 in range(0, nch, self.GMAX):
                        cn = min(self.GMAX, nch - c0)
                        self.calls.append((g, b, t, kb, c0, cn))
                    kb += nch
        self.ncalls = len(self.calls)
        # tight per-call num_idxs immediate: the max real count over cores,
        # rounded to 16 (the ucode's idx-unpack cost scales with it)
        self.call_imm = []
        for (g, b, t, kb, c0, cn) in self.calls:
            mx = int(self.counts[:, t, b].max()) - c0 * TILE
            mx = min(max(mx, 1), cn * TILE)
            imm = min(((mx + 15) // 16) * 16, cn * TILE)
            assert imm > (cn - 1) * TILE, (imm, cn)
            self.call_imm.append(imm)

        # column offsets in the concatenated idx / dstoff DRAM buffers
        self.idx_col = []     # [g][b] -> start col in idx buffer (int16, /16 wrap)
        self.ch_col = []      # [g] -> start chunk col in dstoff buffer
        ic = 0
        cc = 0
        for g in range(len(self.groups)):
            self.ch_col.append(cc)
            row = []
            for b in range(self.nblk):
                row.append(ic)
                ic += self.gb_C[g][b] * (TILE // 16)
                cc += self.gb_C[g][b]
            self.idx_col.append(row)
        self.idx_cols = ic
        self.ch_cols = cc

    def core_inputs(self, c):
        """Build idx (int16, -1 pad), dstoff (bf16) and per-call counts."""
        idx = np.full((16, self.idx_cols), -1, dtype=np.int16)
        doff = np.full((128, self.ch_cols), -1.0, dtype=np.float32)
        for g, tiles in enumerate(self.groups):
            ch = self.ch_col[g]
            for b in range(self.nblk):
                icol = self.idx_col[g][b]
                for (t, nch) in self.gb_chunks[g][b]:
                    cnt = int(self.counts[c, t, b])
                    o = self.seg_off[(c * self.nt + t) * self.nblk + b]
                    nslots = nch * TILE
                    a = np.full(nslots, -1, dtype=np.int16)
                    a[:cnt] = (self.pos_s[o:o + cnt] - b * self.blk).astype(np.int16)
                    idx[:, icol:icol + nch * 8] = a.reshape(nch * 8, 16).T
                    dv = np.full(nslots, -1.0, dtype=np.float32)
                    dv[:cnt] = self.doff_s[o:o + cnt].astype(np.float32)
                    doff[:, ch:ch + nch] = dv.reshape(nch, 128).T
                    icol += nch * 8
                    ch += nch
        idx_full = np.tile(idx, (8, 1))
        cnts = np.zeros((1, self.ncalls), dtype=np.int32)
        for i, (g, b, t, kb, c0, cn) in enumerate(self.calls):
            cnt = int(self.counts[c, t, b])
            cnts[0, i] = min(max(cnt - c0 * TILE, 0), cn * TILE)
        return idx_full, doff.astype(NPBF), cnts


def _build(plan):
    """Build the SPMD bass program (shared by all 8 cores)."""
    n, ns, nt, nblk, blk = plan.n, plan.ns, plan.nt, plan.nblk, plan.blk
    nc = bacc.Bacc("TRN2", target_bir_lowering=False, debug=False,
                   num_devices=NCORES, num_swdge_queues=NQ)

    I32 = mybir.dt.int32
    xT = nc.dram_tensor("xT", [D, ns], BF16, kind="ExternalInput").ap()
    cnts_d = nc.dram_tensor("cnts", [1, plan.ncalls], I32,
                            kind="ExternalInput").ap()
    wts = nc.dram_tensor("wts", [D, 2 * D], BF16, kind="ExternalInput").ap()
    consts = nc.dram_tensor("consts", [D, 2 * D], BF16, kind="ExternalInput").ap()
    brow = nc.dram_tensor("brow", [1, 2 * D], BF16, kind="ExternalInput").ap()
    dinv_c = nc.dram_tensor("dinv_c", [D, nt], F32, kind="ExternalInput").ap()
    sdeg_r = nc.dram_tensor("sdeg_r", [1, nt * TILE], BF16, kind="ExternalInput").ap()
    idx_d = nc.dram_tensor("idx", [D, plan.idx_cols], I16, kind="ExternalInput").ap()
    doff_d = nc.dram_tensor("doff", [D, plan.ch_cols], BF16, kind="ExternalInput").ap()
    out_d = nc.dram_tensor("out", [ns, D], F32, kind="ExternalOutput").ap()

    # tile-major storage, split in two tile-halves per layer; each half is
    # dumped contiguously and AllGather'd separately (overlaps compute)
    nth = plan.nth
    hbh = [[nc.dram_tensor(f"h{i}b{h}", [TILE, nth * D], BF16).ap()
            for h in range(2)] for i in range(2)]
    hfh = [[nc.dram_tensor(f"h{i}f{h}", [plan.NH, D], BF16,
                           addr_space="Shared").ap()
            for h in range(2)] for i in range(2)]

    max_C = max(sum(plan.gb_C[g]) for g in range(len(plan.groups)))
    max_icols = max(sum(plan.gb_C[g]) * 8 for g in range(len(plan.groups)))

    with tile.TileContext(nc) as tc:
        with (
            tc.tile_pool(name="const", bufs=1) as cpool,
            tc.tile_pool(name="xstream", bufs=3) as xpool,
            tc.tile_pool(name="stage", bufs=10) as spool,
            tc.tile_pool(name="oh", bufs=3) as ohpool,
            tc.tile_pool(name="aux", bufs=6) as auxpool,
            tc.tile_pool(name="ev", bufs=4) as evpool,
            tc.tile_pool(name="acc", bufs=4, space="PSUM") as accpool,
            tc.tile_pool(name="ptr", bufs=2, space="PSUM") as trpool,
            tc.tile_pool(name="pd", bufs=2, space="PSUM") as pdpool,
        ):
            w_sb = cpool.tile([D, 2 * D], BF16, tag="w")
            nc.sync.dma_start(out=w_sb[:], in_=wts[:])
            co_sb = cpool.tile([D, 2 * D], BF16, tag="co")
            nc.sync.dma_start(out=co_sb[:], in_=consts[:])
            br_sb = cpool.tile([1, 2 * D], BF16, tag="br")
            nc.sync.dma_start(out=br_sb[:], in_=brow[:])
            dv_sb = cpool.tile([D, nt], F32, tag="dv")
            nc.sync.dma_start(out=dv_sb[:], in_=dinv_c[:])
            sd_sb = cpool.tile([1, nt * TILE], BF16, tag="sd")
            nc.sync.dma_start(out=sd_sb[:], in_=sdeg_r[:])
            cn_sb = cpool.tile([1, plan.ncalls], mybir.dt.int32, tag="cn")
            nc.sync.dma_start(out=cn_sb[:], in_=cnts_d[:])
            # local h' tiles, resident for the diagonal (self-loop) matmul
            hl = [cpool.tile([TILE, nt, D], BF16, tag=f"hl{i}", name=f"hl{i}")
                  for i in range(2)]
            gregs = [nc.gpsimd.alloc_register(f"gcnt{i}") for i in range(NQ)]

            W1 = w_sb[:, 0:D]
            W2 = w_sb[:, D:2 * D]
            iota = co_sb[:, 0:D]
            ident = co_sb[:, D:2 * D]

            def tw(t):
                return TILE if t < nt - 1 else plan.last_w

            def dump_half(li, h):
                """hl[li] tile-half h -> hbh[li][h], then AllGather it."""
                nc.sync.dma_start(
                    out=hbh[li][h][:],
                    in_=hl[li][:, h * nth:(h + 1) * nth, :].opt())
                nc.gpsimd.collective_compute(
                    "AllGather", ALU.bypass,
                    replica_groups=[list(range(NCORES))],
                    ins=[hbh[li][h].opt()], outs=[hfh[li][h].opt()])

            # ---- layer-1 dense: h0' = (x @ W1) * dinv ----
            SLAB = 8
            for s in range(0, nt, SLAB):
                sn = min(SLAB, nt - s)
                sw = (sn - 1) * TILE + tw(s + sn - 1)
                xt_t = xpool.tile([D, SLAB * TILE], BF16, tag="xt")
                nc.sync.dma_start(out=xt_t[:, :sw],
                                  in_=xT[:, s * TILE:s * TILE + sw])
                for j in range(sn):
                    t = s + j
                    w = tw(t)
                    pd = pdpool.tile([TILE, D], F32, tag="pd")
                    nc.tensor.matmul(pd[:w, :],
                                     lhsT=xt_t[:, j * TILE:j * TILE + w],
                                     rhs=W1, start=True, stop=True)
                    # DVE eviction: ~2x faster than ACT here, and the DVE is
                    # idle during the dense phase (gets AG1a fired earlier)
                    nc.vector.tensor_tensor(
                        out=hl[0][:w, t, :], in0=pd[:w, :],
                        in1=dv_sb[:w, t:t + 1].broadcast_to([w, D]),
                        op=ALU.mult)
                if s <= nth - 1 < s + sn:
                    dump_half(0, 0)   # first tile-half done -> AG overlaps rest
            dump_half(0, 1)

            # ---- sparse layer (templated over layer index) ----
            max_Cgb = max((plan.gb_C[g][b] for g in range(len(plan.groups))
                           for b in range(nblk)), default=1)

            # zero all stage buffers once: trimmed gathers leave untouched
            # slots whose virgin SBUF content may be NaN bit patterns, and
            # the PE turns 0 * NaN into NaN despite the zero one-hot column
            for _i in range(10):
                stg0 = spool.tile([D, max_Cgb, TILE], BF16, tag="stage",
                                  name="stg")
                nc.vector.memset(stg0[:].opt(), 0.0)

            qctr = [0]

            # per (g, b) -> list of (t, kb, c0, cn, call_idx)
            bucket_calls = {}
            for ci, (g, b, t, kb, c0, cn) in enumerate(plan.calls):
                bucket_calls.setdefault((g, b), []).append((t, kb, c0, cn, ci))

            # group index after which hl[li=1] tiles 0..nth-1 are all evicted
            g_half = next(gi for gi, tiles in enumerate(plan.groups)
                          if tiles[-1] >= nth - 1)

            def sparse_layer(li):
                tbl = {}       # g -> (idx_sb, do_sb)
                stgs = {}      # (g, b) -> gathered stage tile
                accs_map = {}  # g -> accs

                def load_tables(g):
                    Ctot = sum(plan.gb_C[g])
                    icols = Ctot * 8
                    idx_sb = auxpool.tile([D, max_icols], I16, tag="idx",
                                          name="idx_sb")
                    nc.sync.dma_start(
                        out=idx_sb[:, :icols],
                        in_=idx_d[:, plan.idx_col[g][0]:plan.idx_col[g][0] + icols])
                    do_sb = auxpool.tile([D, max_C], BF16, tag="doff",
                                         name="do_sb")
                    nc.sync.dma_start(
                        out=do_sb[:, :Ctot],
                        in_=doff_d[:, plan.ch_col[g]:plan.ch_col[g] + Ctot])
                    tbl[g] = (idx_sb, do_sb)

                def init_accs(g):
                    accs = {}
                    for t in plan.groups[g]:
                        w = tw(t)
                        accs[t] = accpool.tile([TILE, D], F32, tag="acc", name=f"acc_t{t}")
                        # diagonal (self-loop) term: acc = I @ h'[tile]
                        nc.tensor.matmul(
                            accs[t][:w, :], lhsT=ident[:w, :w],
                            rhs=hl[li][:w, t, :], start=True, stop=False)
                    accs_map[g] = accs

                def do_gather(g, b):
                    idx_sb, _ = tbl[g]
                    Cgb = plan.gb_C[g][b]
                    if Cgb == 0:
                        return
                    ic0 = plan.idx_col[g][b] - plan.idx_col[g][0]
                    stg = spool.tile([D, max_Cgb, TILE], BF16, tag="stage",
                                     name="stg")
                    src_half = hfh[li][b // 2]
                    sbase = (b % 2) * blk
                    for (t, kb, c0, cn, ci) in bucket_calls[(g, b)]:
                        q = qctr[0] % NQ
                        nc.gpsimd.reg_load(gregs[q], cn_sb[0:1, ci:ci + 1])
                        nc.gpsimd.dma_gather(
                            stg[:, kb + c0:kb + c0 + cn, :],
                            src_half[sbase:sbase + blk, :],
                            idx_sb[:, ic0 + (kb + c0) * 8:
                                   ic0 + (kb + c0 + cn) * 8],
                            plan.call_imm[ci],
                            gregs[q],
                            D,
                            queue_num=q,
                        )
                        qctr[0] += 1
                    stgs[(g, b)] = stg

                def do_mm(g, b):
                    Cgb = plan.gb_C[g][b]
                    if Cgb == 0:
                        return
                    _, do_sb = tbl[g]
                    accs = accs_map[g]
                    stg = stgs.pop((g, b))
                    gco = sum(plan.gb_C[g][:b])
                    oh_sb = ohpool.tile([D, max_Cgb, TILE], BF16, tag="oh",
                                        name="oh_sb")
                    nc.vector.scalar_tensor_tensor(
                        out=oh_sb[:, :Cgb, :],
                        in0=do_sb[:, gco:gco + Cgb].unsqueeze(2)
                            .broadcast_to([D, Cgb, TILE]),
                        scalar=1.0,
                        in1=iota.unsqueeze(1).broadcast_to([D, Cgb, TILE]),
                        op0=ALU.mult,
                        op1=ALU.is_equal,
                    )
                    k = 0
                    for (t, nch) in plan.gb_chunks[g][b]:
                        for _ in range(nch):
                            nc.tensor.matmul(
                                accs[t][:], lhsT=oh_sb[:, k, :],
                                rhs=stg[:, k, :],
                                start=False, stop=False)
                            k += 1

                def finish_group(g, li):
                    tiles = plan.groups[g]
                    accs = accs_map[g]
                    for t in tiles:
                        w = tw(t)
                        acc = accs[t]
                        # bias as rank-1: outer(sqrt(deg), b); sdeg rows
                        # beyond the tile width are zero-padded on the host
                        nc.tensor.matmul(
                            acc[:],
                            lhsT=sd_sb[:, t * TILE:(t + 1) * TILE],
                            rhs=br_sb[:, li * D:(li + 1) * D],
                            start=False, stop=True)
                        if li == 0:
                            ev = evpool.tile([TILE, D], BF16, tag="ev")
                            nc.scalar.activation(ev[:w, :], acc[:w, :], AF.Copy,
                                                 scale=dv_sb[:w, t:t + 1])
                            # fused layer-2 dense: h1' = (out1 @ W2) * dinv
                            ptr = trpool.tile([D, TILE], BF16, tag="ptr")
                            nc.tensor.transpose(ptr[:, :w], ev[:w, :],
                                                ident[:w, :w])
                            trs = evpool.tile([D, TILE], BF16, tag="trs")
                            nc.vector.tensor_copy(trs[:, :w], ptr[:, :w])
                            pd = pdpool.tile([TILE, D], F32, tag="pd")
                            nc.tensor.matmul(pd[:w, :], lhsT=trs[:, :w], rhs=W2,
                                             start=True, stop=True)
                            nc.scalar.activation(hl[1][:w, t, :], pd[:w, :],
                                                 AF.Copy,
                                                 scale=dv_sb[:w, t:t + 1])
                        else:
                            evf = evpool.tile([TILE, D], F32, tag="evf")
                            nc.scalar.activation(evf[:w, :], acc[:w, :], AF.Copy,
                                                 scale=dv_sb[:w, t:t + 1])
                            nc.sync.dma_start(
                                out=out_d[t * TILE:t * TILE + w, :],
                                in_=evf[:w, :])

                    del accs_map[g]
                    if li == 0 and g == g_half:
                        # first tile-half of h1' is complete: AG it now so it
                        # overlaps the rest of sparse layer 1
                        dump_half(1, 0)

                # first K groups: gather blocks 0-1 (first-half AG) before any
                # block 2-3 gather, so the second-half AG completes under work
                ngroups = len(plan.groups)
                K = min(4, ngroups)
                for g in range(K):
                    load_tables(g)
                    do_gather(g, 0)
                    do_gather(g, 1)
                for g in range(K):
                    init_accs(g)
                    do_mm(g, 0)
                    do_mm(g, 1)
                    do_gather(g, 2)
                    do_mm(g, 2)
                    do_gather(g, 3)
                    do_mm(g, 3)
                    finish_group(g, li)
                for g in range(K, ngroups):
                    load_tables(g)
                    init_accs(g)
                    for b in range(nblk):
                        do_gather(g, b)
                        do_mm(g, b)
                    finish_group(g, li)

            sparse_layer(0)
            dump_half(1, 1)
            sparse_layer(1)

    nc.compile()
    return nc


def _install_ntff_hook():
    """antenv.axon_hooks is absent in this image; synthesize it and register
    the ctypes NTFF profile hook from the boot module."""
    import types
    if "antenv.axon_hooks" in sys.modules:
        return
    try:
        from trn_agent_boot.trn_boot import _ntff_profile_via_ctypes
        hook = _ntff_profile_via_ctypes("/opt/axon/libaxon_pjrt.so")
    except Exception as e:
        print(f"[kernel] ntff hook unavailable: {e}", flush=True)
        hook = None
    mod = types.ModuleType("antenv.axon_hooks")
    mod._hook = hook
    mod.set_axon_ntff_profile_hook = lambda h: setattr(mod, "_hook", h)
    mod.get_axon_ntff_profile_hook = lambda: mod._hook
    sys.modules["antenv.axon_hooks"] = mod
    import antenv
    antenv.axon_hooks = mod


def _run(plan, x, W1, b1, W2, b2, trace=False, stage="full"):
    import time
    if trace:
        _install_ntff_hook()
    t0 = time.time()
    nc = _build(plan)
    t1 = time.time()
    if os.environ.get("GCN_VERBOSE"):
        print(f"[kernel] build+compile: {t1 - t0:.1f}s", flush=True)
    ns, nt = plan.ns, plan.nt
    iota_t = np.tile(np.arange(TILE, dtype=np.float32), (TILE, 1))
    ident_t = np.eye(TILE, dtype=np.float32)
    consts = np.concatenate([iota_t, ident_t], axis=1).astype(NPBF)
    wts = np.concatenate([W1.astype(np.float32), W2.astype(np.float32)],
                         axis=1).astype(NPBF)
    brow = np.concatenate([b1.astype(np.float32), b2.astype(np.float32)]
                          ).reshape(1, 2 * D).astype(NPBF)

    in_maps = []
    for c in range(NCORES):
        lo, hi = c * ns, (c + 1) * ns
        dv = plan.dinv[lo:hi]
        # column t of dcol holds dinv[lo + t*128 : lo + (t+1)*128] (pad 1.0)
        dcol = np.ones((nt, TILE), dtype=np.float32)
        dcol.reshape(-1)[:ns] = dv
        dcol = np.ascontiguousarray(dcol.T)
        sdr = np.zeros((1, nt * TILE), dtype=np.float32)
        sdr[0, :ns] = plan.sdeg[lo:hi]
        idx, doff, cnts = plan.core_inputs(c)
        in_maps.append({
            "xT": np.ascontiguousarray(x[lo:hi].astype(np.float32).T).astype(NPBF),
            "wts": wts, "consts": consts, "brow": brow,
            "dinv_c": dcol, "sdeg_r": sdr.astype(NPBF),
            "idx": idx, "doff": doff, "cnts": cnts,
        })
    t2 = time.time()
    res = run_bass_kernel_spmd(nc, in_maps, core_ids=list(range(NCORES)),
                               trace=trace)
    if os.environ.get("GCN_VERBOSE"):
        print(f"[kernel] prep inputs: {t2 - t1:.1f}s, run: {time.time() - t2:.1f}s",
              flush=True)
    out = np.concatenate([res.results[c]["out"] for c in range(NCORES)], axis=0)
    return out, res


def kernel(x, edge_index, W1, b1, W2, b2):
    plan = Plan(x.shape[0], np.asarray(edge_index))
    out, _ = _run(plan, np.asarray(x), np.asarray(W1), np.asarray(b1),
                  np.asarray(W2), np.asarray(b2))
    return out


# revision 66
# speedup vs baseline: 1.0316x; 1.0214x over previous
"""Trainium2 Bass kernel: 2-layer GCN (PyG-style GCNConv x2) on 8 NeuronCores.

Strategy:
  - Nodes sharded contiguously across 8 cores (12500 rows each).
  - Per layer: dense h' = (x @ W) * dinv[row] computed on the owning core
    (bf16) into a resident SBUF tile (hl), dumped to DRAM in one contiguous
    DMA (tile-major layout), AllGather'd to every core (25.6MB replica),
    then per-core sparse aggregation over its in-edges:
      gather h'[src] rows via dma_gather (int16 idx into 4 blocks of 25088
      storage positions; 4 SWDGE queues round-robin so Q7 descriptor
      emission runs on all four core pairs concurrently; per-core exact
      counts via num_idxs_reg + trailing -1 idx so padding is never
      gathered; per-bucket idx sorted ascending for HBM row locality),
      scatter-add via one-hot matmul into PSUM per 128-dst tile,
      self-loops as a diagonal identity matmul from the resident hl tile,
      bias added as rank-1 matmul outer(sqrt(deg), b),
      eviction scaled by dinv[dst] on the scalar engine (layer 1 fuses the
      layer-2 dense transform behind a PE transpose).
  - The per-edge norm dinv[src]*dinv[dst] is folded into the two node-level
    scalings, so no per-edge multiply exists anywhere.
  - All matmul/gather traffic in bf16 (tolerance 2e-2 >> bf16 error);
    PSUM accumulation fp32; final output fp32.
"""

import os
import sys

for _p in ("/opt/trn_rl_repo",):
    if _p not in sys.path:
        sys.path.append(_p)

import numpy as np
import ml_dtypes

import concourse.bacc as bacc
import concourse.bass as bass
import concourse.mybir as mybir
import concourse.tile as tile
from concourse.bass_utils import run_bass_kernel_spmd

F32 = mybir.dt.float32
BF16 = mybir.dt.bfloat16
I16 = mybir.dt.int16
AF = mybir.ActivationFunctionType
ALU = mybir.AluOpType
NPBF = ml_dtypes.bfloat16

N_NODES = 100000
D = 128
NCORES = 8
TILE = 128
NQ = 4  # SWDGE queues round-robin for dma_gather descriptor emission


def _ceil_div(a, b):
    return (a + b - 1) // b


class Plan:
    """Core-uniform structure tables derived from the edge index."""

    def __init__(self, n_nodes, edge_index, group_tiles=4):
        self.n = n_nodes
        self.ns = n_nodes // NCORES            # nodes per core
        self.nt = _ceil_div(self.ns, TILE)     # dst tiles per core
        self.last_w = self.ns - (self.nt - 1) * TILE
        self.G = group_tiles
        # h' is stored tile-major per core and AllGather'd in 3 tile-pieces
        # (49, 25, 24): piece 0 fires mid-dense / mid-sparse, pieces 1-2 let
        # the tail AGs trigger earlier than a single second-half would.
        # pos of node (c, t, rr): pbase[p] + (c*128+rr)*pt[p] + (t-pstart[p])
        p0 = self.nt // 2
        p1 = (self.nt - p0 + 1) // 2
        self.pt = [p0, p1, self.nt - p0 - p1]
        self.pstart = [0, p0, p0 + p1]
        self.psize = [NCORES * TILE * t for t in self.pt]
        self.pbase = [sum(self.psize[:p]) for p in range(3)]
        self.nblk = 4                          # src blocks (int16 idx limit)
        # blocks 0,1 split piece 0; block 2 = piece 1; block 3 = piece 2
        self.bbase = [0, self.psize[0] // 2, self.pbase[1], self.pbase[2]]
        self.bsize = [self.psize[0] // 2, self.psize[0] // 2,
                      self.psize[1], self.psize[2]]
        assert max(self.bsize) <= 32768

        # degree includes self-loops (PyG GCNConv semantics) but the self
        # edges are NOT gathered: handled as a diagonal identity matmul.
        deg = np.bincount(edge_index[1], minlength=n_nodes).astype(np.float32)
        deg += 1.0
        self.dinv = deg ** -0.5
        self.sdeg = np.sqrt(deg)
        src = np.asarray(edge_index[0])
        dst = np.asarray(edge_index[1])

        core = dst // self.ns
        tloc = (dst % self.ns) // TILE
        sc = src // self.ns
        sr = src % self.ns
        st = sr // TILE
        pof = np.zeros(self.nt, np.int64)
        for p in range(3):
            pof[self.pstart[p]:self.pstart[p] + self.pt[p]] = p
        pp = pof[st]
        pos = (np.asarray(self.pbase)[pp]
               + (sc * TILE + (sr % TILE)) * np.asarray(self.pt)[pp]
               + (st - np.asarray(self.pstart)[pp]))
        blk = np.searchsorted(np.asarray(self.bbase), pos, side='right') - 1
        key = (core * self.nt + tloc) * self.nblk + blk
        # ascending storage pos within each (core,tile,blk) bucket: the
        # per-bucket gather descriptors then sweep HBM monotonically
        order = np.lexsort((pos, key))
        self.pos_s = pos[order]
        self.doff_s = ((dst % self.ns) % TILE)[order]
        counts = np.bincount(key, minlength=NCORES * self.nt * self.nblk)
        self.counts = counts.reshape(NCORES, self.nt, self.nblk)
        # segment start offsets into src_s per (core, tile, blk)
        self.seg_off = np.zeros(NCORES * self.nt * self.nblk + 1, dtype=np.int64)
        np.cumsum(counts, out=self.seg_off[1:])

        # chunks per (tile, blk): shared across cores
        self.CT = _ceil_div(self.counts, TILE).max(axis=0)  # [nt, nblk]

        # tile groups
        self.groups = [list(range(g, min(g + self.G, self.nt)))
                       for g in range(0, self.nt, self.G)]

        # per (group, blk): chunk count and the (tile, n_chunks) layout
        self.gb_chunks = []   # [g][b] -> list of (tile, CT[t][b])
        self.gb_C = []        # [g][b] -> total chunks
        for tiles in self.groups:
            row_l, row_c = [], []
            for b in range(self.nblk):
                lay = [(t, int(self.CT[t, b])) for t in tiles if self.CT[t, b] > 0]
                row_l.append(lay)
                row_c.append(sum(c for _, c in lay))
            self.gb_chunks.append(row_l)
            self.gb_C.append(row_c)

        # dma_gather call schedule: one call never spans a (tile, blk)
        # segment so per-core trailing -1 idx trimming (num_idxs_reg) works
        self.GMAX = 8
        self.calls = []   # (g, b, t, kb: chunk col in bucket, c0: within seg, cn)
        for g in range(len(self.groups)):
            for b in range(self.nblk):
                kb = 0
                for (t, nch) in self.gb_chunks[g][b]:
                    for c0 in range(0, nch, self.GMAX):
                        cn = min(self.GMAX, nch - c0)
                        self.calls.append((g, b, t, kb, c0, cn))
                    kb += nch
        self.ncalls = len(self.calls)
        # tight per-call num_idxs immediate: the max real count over cores,
        # rounded to 16 (the ucode's idx-unpack cost scales with it)
        self.call_imm = []
        for (g, b, t, kb, c0, cn) in self.calls:
            mx = int(self.counts[:, t, b].max()) - c0 * TILE
            mx = min(max(mx, 1), cn * TILE)
            imm = min(((mx + 15) // 16) * 16, cn * TILE)
            assert imm > (cn - 1) * TILE, (imm, cn)
            self.call_imm.append(imm)

        # column offsets in the concatenated idx / dstoff DRAM buffers
        self.idx_col = []     # [g][b] -> start col in idx buffer (int16, /16 wrap)
        self.ch_col = []      # [g] -> start chunk col in dstoff buffer
        ic = 0
        cc = 0
        for g in range(len(self.groups)):
            self.ch_col.append(cc)
            row = []
            for b in range(self.nblk):
                row.append(ic)
                ic += self.gb_C[g][b] * (TILE // 16)
                cc += self.gb_C[g][b]
            self.idx_col.append(row)
        self.idx_cols = ic
        self.ch_cols = cc

    def core_inputs(self, c):
        """Build idx (int16, -1 pad), dstoff (bf16) and per-call counts."""
        idx = np.full((16, self.idx_cols), -1, dtype=np.int16)
        doff = np.full((128, self.ch_cols), -1.0, dtype=np.float32)
        for g, tiles in enumerate(self.groups):
            ch = self.ch_col[g]
            for b in range(self.nblk):
                icol = self.idx_col[g][b]
                for (t, nch) in self.gb_chunks[g][b]:
                    cnt = int(self.counts[c, t, b])
                    o = self.seg_off[(c * self.nt + t) * self.nblk + b]
                    nslots = nch * TILE
                    a = np.full(nslots, -1, dtype=np.int16)
                    a[:cnt] = (self.pos_s[o:o + cnt] - self.bbase[b]).astype(np.int16)
                    idx[:, icol:icol + nch * 8] = a.reshape(nch * 8, 16).T
                    dv = np.full(nslots, -1.0, dtype=np.float32)
                    dv[:cnt] = self.doff_s[o:o + cnt].astype(np.float32)
                    doff[:, ch:ch + nch] = dv.reshape(nch, 128).T
                    icol += nch * 8
                    ch += nch
        idx_full = np.tile(idx, (8, 1))
        cnts = np.zeros((1, self.ncalls), dtype=np.int32)
        for i, (g, b, t, kb, c0, cn) in enumerate(self.calls):
            cnt = int(self.counts[c, t, b])
            cnts[0, i] = min(max(cnt - c0 * TILE, 0), cn * TILE)
        return idx_full, doff.astype(NPBF), cnts


def _build(plan):
    """Build the SPMD bass program (shared by all 8 cores)."""
    n, ns, nt, nblk = plan.n, plan.ns, plan.nt, plan.nblk
    nc = bacc.Bacc("TRN2", target_bir_lowering=False, debug=False,
                   num_devices=NCORES, num_swdge_queues=NQ)

    I32 = mybir.dt.int32
    xT = nc.dram_tensor("xT", [D, ns], BF16, kind="ExternalInput").ap()
    cnts_d = nc.dram_tensor("cnts", [1, plan.ncalls], I32,
                            kind="ExternalInput").ap()
    wts = nc.dram_tensor("wts", [D, 2 * D], BF16, kind="ExternalInput").ap()
    consts = nc.dram_tensor("consts", [D, 2 * D], BF16, kind="ExternalInput").ap()
    brow = nc.dram_tensor("brow", [1, 2 * D], BF16, kind="ExternalInput").ap()
    dinv_c = nc.dram_tensor("dinv_c", [D, nt], F32, kind="ExternalInput").ap()
    sdeg_r = nc.dram_tensor("sdeg_r", [1, nt * TILE], BF16, kind="ExternalInput").ap()
    idx_d = nc.dram_tensor("idx", [D, plan.idx_cols], I16, kind="ExternalInput").ap()
    doff_d = nc.dram_tensor("doff", [D, plan.ch_cols], BF16, kind="ExternalInput").ap()
    out_d = nc.dram_tensor("out", [ns, D], F32, kind="ExternalOutput").ap()

    # tile-major storage, split in 3 tile-pieces per layer; each piece is
    # dumped contiguously and AllGather'd separately (overlaps compute)
    ps_, pt_, psz = plan.pstart, plan.pt, plan.psize
    hbp = [[nc.dram_tensor(f"h{i}b{p}", [TILE, pt_[p] * D], BF16).ap()
            for p in range(3)] for i in range(2)]
    hfp = [[nc.dram_tensor(f"h{i}f{p}", [psz[p], D], BF16,
                           addr_space="Shared").ap()
            for p in range(3)] for i in range(2)]

    max_C = max(sum(plan.gb_C[g]) for g in range(len(plan.groups)))
    max_icols = max(sum(plan.gb_C[g]) * 8 for g in range(len(plan.groups)))

    with tile.TileContext(nc) as tc:
        with (
            tc.tile_pool(name="const", bufs=1) as cpool,
            tc.tile_pool(name="xstream", bufs=3) as xpool,
            tc.tile_pool(name="stage", bufs=10) as spool,
            tc.tile_pool(name="oh", bufs=3) as ohpool,
            tc.tile_pool(name="aux", bufs=6) as auxpool,
            tc.tile_pool(name="ev", bufs=4) as evpool,
            tc.tile_pool(name="acc", bufs=4, space="PSUM") as accpool,
            tc.tile_pool(name="ptr", bufs=2, space="PSUM") as trpool,
            tc.tile_pool(name="pd", bufs=2, space="PSUM") as pdpool,
        ):
            w_sb = cpool.tile([D, 2 * D], BF16, tag="w")
            nc.sync.dma_start(out=w_sb[:], in_=wts[:])
            co_sb = cpool.tile([D, 2 * D], BF16, tag="co")
            nc.sync.dma_start(out=co_sb[:], in_=consts[:])
            br_sb = cpool.tile([1, 2 * D], BF16, tag="br")
            nc.sync.dma_start(out=br_sb[:], in_=brow[:])
            dv_sb = cpool.tile([D, nt], F32, tag="dv")
            nc.sync.dma_start(out=dv_sb[:], in_=dinv_c[:])
            sd_sb = cpool.tile([1, nt * TILE], BF16, tag="sd")
            nc.sync.dma_start(out=sd_sb[:], in_=sdeg_r[:])
            cn_sb = cpool.tile([1, plan.ncalls], mybir.dt.int32, tag="cn")
            nc.sync.dma_start(out=cn_sb[:], in_=cnts_d[:])
            # local h' tiles, resident for the diagonal (self-loop) matmul
            hl = [cpool.tile([TILE, nt, D], BF16, tag=f"hl{i}", name=f"hl{i}")
                  for i in range(2)]
            gregs = [nc.gpsimd.alloc_register(f"gcnt{i}") for i in range(NQ)]

            W1 = w_sb[:, 0:D]
            W2 = w_sb[:, D:2 * D]
            iota = co_sb[:, 0:D]
            ident = co_sb[:, D:2 * D]

            def tw(t):
                return TILE if t < nt - 1 else plan.last_w

            def dump_piece(li, p):
                """hl[li] tile-piece p -> hbp[li][p], then AllGather it."""
                nc.sync.dma_start(
                    out=hbp[li][p][:],
                    in_=hl[li][:, ps_[p]:ps_[p] + pt_[p], :].opt())
                nc.gpsimd.collective_compute(
                    "AllGather", ALU.bypass,
                    replica_groups=[list(range(NCORES))],
                    ins=[hbp[li][p].opt()], outs=[hfp[li][p].opt()])

            # ---- layer-1 dense: h0' = (x @ W1) * dinv ----
            SLAB = 8
            for s in range(0, nt, SLAB):
                sn = min(SLAB, nt - s)
                sw = (sn - 1) * TILE + tw(s + sn - 1)
                xt_t = xpool.tile([D, SLAB * TILE], BF16, tag="xt")
                nc.sync.dma_start(out=xt_t[:, :sw],
                                  in_=xT[:, s * TILE:s * TILE + sw])
                for j in range(sn):
                    t = s + j
                    w = tw(t)
                    pd = pdpool.tile([TILE, D], F32, tag="pd")
                    nc.tensor.matmul(pd[:w, :],
                                     lhsT=xt_t[:, j * TILE:j * TILE + w],
                                     rhs=W1, start=True, stop=True)
                    nc.scalar.activation(hl[0][:w, t, :], pd[:w, :], AF.Copy,
                                         scale=dv_sb[:w, t:t + 1])
                for p in range(2):
                    if s <= ps_[p] + pt_[p] - 1 < s + sn:
                        dump_piece(0, p)  # piece done -> AG overlaps rest
            dump_piece(0, 2)

            # ---- sparse layer (templated over layer index) ----
            max_Cgb = max((plan.gb_C[g][b] for g in range(len(plan.groups))
                           for b in range(nblk)), default=1)

            # zero all stage buffers once: trimmed gathers leave untouched
            # slots whose virgin SBUF content may be NaN bit patterns, and
            # the PE turns 0 * NaN into NaN despite the zero one-hot column
            for _i in range(10):
                stg0 = spool.tile([D, max_Cgb, TILE], BF16, tag="stage",
                                  name="stg")
                nc.vector.memset(stg0[:].opt(), 0.0)

            qctr = [0]

            # per (g, b) -> list of (t, kb, c0, cn, call_idx)
            bucket_calls = {}
            for ci, (g, b, t, kb, c0, cn) in enumerate(plan.calls):
                bucket_calls.setdefault((g, b), []).append((t, kb, c0, cn, ci))

            # group index after which hl[li=1] piece p is fully evicted
            phooks = {}
            for p in range(2):
                plast = ps_[p] + pt_[p] - 1
                gi = next(gi for gi, tiles in enumerate(plan.groups)
                          if tiles[-1] >= plast)
                phooks[gi] = p

            def sparse_layer(li):
                tbl = {}       # g -> (idx_sb, do_sb)
                stgs = {}      # (g, b) -> gathered stage tile
                accs_map = {}  # g -> accs

                def load_tables(g):
                    Ctot = sum(plan.gb_C[g])
                    icols = Ctot * 8
                    idx_sb = auxpool.tile([D, max_icols], I16, tag="idx",
                                          name="idx_sb")
                    nc.sync.dma_start(
                        out=idx_sb[:, :icols],
                        in_=idx_d[:, plan.idx_col[g][0]:plan.idx_col[g][0] + icols])
                    do_sb = auxpool.tile([D, max_C], BF16, tag="doff",
                                         name="do_sb")
                    nc.sync.dma_start(
                        out=do_sb[:, :Ctot],
                        in_=doff_d[:, plan.ch_col[g]:plan.ch_col[g] + Ctot])
                    tbl[g] = (idx_sb, do_sb)

                def init_accs(g):
                    accs = {}
                    for t in plan.groups[g]:
                        w = tw(t)
                        accs[t] = accpool.tile([TILE, D], F32, tag="acc", name=f"acc_t{t}")
                        # diagonal (self-loop) term: acc = I @ h'[tile]
                        nc.tensor.matmul(
                            accs[t][:w, :], lhsT=ident[:w, :w],
                            rhs=hl[li][:w, t, :], start=True, stop=False)
                    accs_map[g] = accs

                def do_gather(g, b):
                    idx_sb, _ = tbl[g]
                    Cgb = plan.gb_C[g][b]
                    if Cgb == 0:
                        return
                    ic0 = plan.idx_col[g][b] - plan.idx_col[g][0]
                    stg = spool.tile([D, max_Cgb, TILE], BF16, tag="stage",
                                     name="stg")
                    src_p = hfp[li][0 if b < 2 else b - 1]
                    sbase = plan.bbase[b] - plan.pbase[0 if b < 2 else b - 1]
                    for (t, kb, c0, cn, ci) in bucket_calls[(g, b)]:
                        q = qctr[0] % NQ
                        nc.gpsimd.reg_load(gregs[q], cn_sb[0:1, ci:ci + 1])
                        nc.gpsimd.dma_gather(
                            stg[:, kb + c0:kb + c0 + cn, :],
                            src_p[sbase:sbase + plan.bsize[b], :],
                            idx_sb[:, ic0 + (kb + c0) * 8:
                                   ic0 + (kb + c0 + cn) * 8],
                            plan.call_imm[ci],
                            gregs[q],
                            D,
                            queue_num=q,
                        )
                        qctr[0] += 1
                    stgs[(g, b)] = stg

                def do_mm(g, b):
                    Cgb = plan.gb_C[g][b]
                    if Cgb == 0:
                        return
                    _, do_sb = tbl[g]
                    accs = accs_map[g]
                    stg = stgs.pop((g, b))
                    gco = sum(plan.gb_C[g][:b])
                    oh_sb = ohpool.tile([D, max_Cgb, TILE], BF16, tag="oh",
                                        name="oh_sb")
                    nc.vector.scalar_tensor_tensor(
                        out=oh_sb[:, :Cgb, :],
                        in0=do_sb[:, gco:gco + Cgb].unsqueeze(2)
                            .broadcast_to([D, Cgb, TILE]),
                        scalar=1.0,
                        in1=iota.unsqueeze(1).broadcast_to([D, Cgb, TILE]),
                        op0=ALU.mult,
                        op1=ALU.is_equal,
                    )
                    k = 0
                    for (t, nch) in plan.gb_chunks[g][b]:
                        for _ in range(nch):
                            nc.tensor.matmul(
                                accs[t][:], lhsT=oh_sb[:, k, :],
                                rhs=stg[:, k, :],
                                start=False, stop=False)
                            k += 1

                def finish_group(g, li):
                    tiles = plan.groups[g]
                    accs = accs_map[g]
                    for t in tiles:
                        w = tw(t)
                        acc = accs[t]
                        # bias as rank-1: outer(sqrt(deg), b); sdeg rows
                        # beyond the tile width are zero-padded on the host
                        nc.tensor.matmul(
                            acc[:],
                            lhsT=sd_sb[:, t * TILE:(t + 1) * TILE],
                            rhs=br_sb[:, li * D:(li + 1) * D],
                            start=False, stop=True)
                        if li == 0:
                            ev = evpool.tile([TILE, D], BF16, tag="ev")
                            nc.scalar.activation(ev[:w, :], acc[:w, :], AF.Copy,
                                                 scale=dv_sb[:w, t:t + 1])
                            # fused layer-2 dense: h1' = (out1 @ W2) * dinv
                            ptr = trpool.tile([D, TILE], BF16, tag="ptr")
                            nc.tensor.transpose(ptr[:, :w], ev[:w, :],
                                                ident[:w, :w])
                            trs = evpool.tile([D, TILE], BF16, tag="trs")
                            nc.vector.tensor_copy(trs[:, :w], ptr[:, :w])
                            pd = pdpool.tile([TILE, D], F32, tag="pd")
                            nc.tensor.matmul(pd[:w, :], lhsT=trs[:, :w], rhs=W2,
                                             start=True, stop=True)
                            nc.scalar.activation(hl[1][:w, t, :], pd[:w, :],
                                                 AF.Copy,
                                                 scale=dv_sb[:w, t:t + 1])
                        else:
                            evf = evpool.tile([TILE, D], F32, tag="evf")
                            nc.scalar.activation(evf[:w, :], acc[:w, :], AF.Copy,
                                                 scale=dv_sb[:w, t:t + 1])
                            nc.sync.dma_start(
                                out=out_d[t * TILE:t * TILE + w, :],
                                in_=evf[:w, :])

                    del accs_map[g]
                    if li == 0 and g in phooks:
                        # a tile-piece of h1' is complete: AG it now so it
                        # overlaps the rest of sparse layer 1
                        dump_piece(1, phooks[g])

                # first K groups: gather blocks 0-1 (first-half AG) before any
                # block 2-3 gather, so the second-half AG completes under work
                ngroups = len(plan.groups)
                K = min(4, ngroups)
                for g in range(K):
                    load_tables(g)
                    do_gather(g, 0)
                    do_gather(g, 1)
                for g in range(K):
                    init_accs(g)
                    do_mm(g, 0)
                    do_mm(g, 1)
                    do_gather(g, 2)
                    do_mm(g, 2)
                    do_gather(g, 3)
                    do_mm(g, 3)
                    finish_group(g, li)
                for g in range(K, ngroups):
                    load_tables(g)
                    init_accs(g)
                    for b in range(nblk):
                        do_gather(g, b)
                        do_mm(g, b)
                    finish_group(g, li)

            sparse_layer(0)
            dump_piece(1, 2)
            sparse_layer(1)

    nc.compile()
    return nc


def _install_ntff_hook():
    """antenv.axon_hooks is absent in this image; synthesize it and register
    the ctypes NTFF profile hook from the boot module."""
    import types
    if "antenv.axon_hooks" in sys.modules:
        return
    try:
        from trn_agent_boot.trn_boot import _ntff_profile_via_ctypes
        hook = _ntff_profile_via_ctypes("/opt/axon/libaxon_pjrt.so")
    except Exception as e:
        print(f"[kernel] ntff hook unavailable: {e}", flush=True)
        hook = None
    mod = types.ModuleType("antenv.axon_hooks")
    mod._hook = hook
    mod.set_axon_ntff_profile_hook = lambda h: setattr(mod, "_hook", h)
    mod.get_axon_ntff_profile_hook = lambda: mod._hook
    sys.modules["antenv.axon_hooks"] = mod
    import antenv
    antenv.axon_hooks = mod


def _run(plan, x, W1, b1, W2, b2, trace=False, stage="full"):
    import time
    if trace:
        _install_ntff_hook()
    t0 = time.time()
    nc = _build(plan)
    t1 = time.time()
    if os.environ.get("GCN_VERBOSE"):
        print(f"[kernel] build+compile: {t1 - t0:.1f}s", flush=True)
    ns, nt = plan.ns, plan.nt
    iota_t = np.tile(np.arange(TILE, dtype=np.float32), (TILE, 1))
    ident_t = np.eye(TILE, dtype=np.float32)
    consts = np.concatenate([iota_t, ident_t], axis=1).astype(NPBF)
    wts = np.concatenate([W1.astype(np.float32), W2.astype(np.float32)],
                         axis=1).astype(NPBF)
    brow = np.concatenate([b1.astype(np.float32), b2.astype(np.float32)]
                          ).reshape(1, 2 * D).astype(NPBF)

    in_maps = []
    for c in range(NCORES):
        lo, hi = c * ns, (c + 1) * ns
        dv = plan.dinv[lo:hi]
        # column t of dcol holds dinv[lo + t*128 : lo + (t+1)*128] (pad 1.0)
        dcol = np.ones((nt, TILE), dtype=np.float32)
        dcol.reshape(-1)[:ns] = dv
        dcol = np.ascontiguousarray(dcol.T)
        sdr = np.zeros((1, nt * TILE), dtype=np.float32)
        sdr[0, :ns] = plan.sdeg[lo:hi]
        idx, doff, cnts = plan.core_inputs(c)
        in_maps.append({
            "xT": np.ascontiguousarray(x[lo:hi].astype(np.float32).T).astype(NPBF),
            "wts": wts, "consts": consts, "brow": brow,
            "dinv_c": dcol, "sdeg_r": sdr.astype(NPBF),
            "idx": idx, "doff": doff, "cnts": cnts,
        })
    t2 = time.time()
    res = run_bass_kernel_spmd(nc, in_maps, core_ids=list(range(NCORES)),
                               trace=trace)
    if os.environ.get("GCN_VERBOSE"):
        print(f"[kernel] prep inputs: {t2 - t1:.1f}s, run: {time.time() - t2:.1f}s",
              flush=True)
    out = np.concatenate([res.results[c]["out"] for c in range(NCORES)], axis=0)
    return out, res


def kernel(x, edge_index, W1, b1, W2, b2):
    plan = Plan(x.shape[0], np.asarray(edge_index))
    out, _ = _run(plan, np.asarray(x), np.asarray(W1), np.asarray(b1),
                  np.asarray(W2), np.asarray(b2))
    return out
